# revision 1
# baseline (speedup 1.0000x reference)
"""Causal self-attention (B=4, T=2048, D=2048, H=16, HD=128) on 8 Trainium2
NeuronCores.

Sharding: Megatron-style tensor parallel over heads for QKV projection +
attention (2 heads per core), then on-device AllToAlls reshard from
head-parallel to token-parallel (core j owns tokens of batch j//2, half j%2)
for the output projection.  Host only slices/transposes weights, replicates
activations, and concatenates the 8 output shards.

Device layouts (all matmul operands bf16, fp32 PSUM accumulation):
  xT    [D, B*T]   x transposed (contraction dim on partitions)
  qT/kT [128, T]   per (local head, batch); d-order permuted so the RoPE
                   rotate-half partner sits 16 partitions away (within a
                   32-partition quadrant, reachable by DVE stream_shuffle).
                   Any consistent permutation of d leaves q.k unchanged.
  V     [T, 128]   natural d order (feeds AV matmul lhsT and out-proj order)
  S^T   [tk, tq]   scores transposed: the softmax sum over the partition dim
                   is a ones-matmul on the PE (output rows are the broadcast
                   sums for free); no max-subtraction needed (logits ~
                   N(0,1), bounded ~ +-6, exp can't overflow).

The attention loop runs tq-half 0 (even 512-token chunks) then half 1, with
one AllToAll per (half, head) issued as soon as that head's chunks finish —
all four collectives overlap the remaining attention / output projection.
"""

import sys

for _p in ("/opt/trn_rl_repo", "/root/.axon_site/_ro/trn_rl_repo"):
    if _p not in sys.path:
        sys.path.insert(0, _p)

import numpy as np
import ml_dtypes

BF16 = ml_dtypes.bfloat16

B = 4
D = 2048
H = 16
HD = 128
NCORES = 8
HL = 2           # heads per core
CB = D // 128    # contraction blocks
TCH = 512        # token chunk (matmul moving free dim)


def _perm128():
    """Partition order for q/k head dims: quadrant g holds dims
    [16g,16g+16) (lo) then [64+16g, 64+16g+16) (hi), so the rotate-half
    partner of partition p is p+-16 (same 32-partition quadrant)."""
    perm = np.zeros(128, np.int64)
    for p in range(128):
        g, i = divmod(p, 32)
        perm[p] = g * 16 + i if i < 16 else 64 + g * 16 + (i - 16)
    return perm


_PERM = _perm128()
_SHUF = [(i + 16) % 32 for i in range(32)]  # out[i] = in[(i+16)%32]
_SIGN = np.where(np.arange(128) % 32 < 16, -1.0, 1.0).astype(np.float32)


def build_nc(T=2048):
    import concourse.bacc as bacc
    import concourse.tile as tile
    import concourse.mybir as mybir

    f32 = mybir.dt.float32
    bf16 = mybir.dt.bfloat16
    TOK = B * T
    THALF = T // 2
    TQ = THALF // 2           # tokens per (core, a2a part)
    NCH = TOK // TCH          # token chunks total
    CHB = T // TCH            # token chunks per batch
    TB = T // 128             # 128-token blocks per batch
    SCALE = float(HD) ** -0.5
    Exp = mybir.ActivationFunctionType.Exp

    assert TQ == TCH, "A2A split layout assumes T == 2048"
    nc = bacc.Bacc("TRN2", target_bir_lowering=False, debug=False,
                   num_devices=NCORES)

    xT_d = nc.dram_tensor("xT", [D, TOK], bf16, kind="ExternalInput")
    wqkT_d = nc.dram_tensor("wqkT", [D, 6 * HD], bf16, kind="ExternalInput")
    woutT_d = nc.dram_tensor("woutT", [D, D], bf16, kind="ExternalInput")
    cosT_d = nc.dram_tensor("cosT", [HD, T], bf16, kind="ExternalInput")
    sinS_d = nc.dram_tensor("sinS", [HD, T], bf16, kind="ExternalInput")
    out_d = nc.dram_tensor("out", [THALF, D], f32, kind="ExternalOutput")

    xT_v = xT_d.ap().rearrange("(cb p) t -> p cb t", p=128)
    wqkT_v = wqkT_d.ap().rearrange("(cb p) f -> p cb f", p=128)
    woutT_v = woutT_d.ap().rearrange("(cb p) o -> p cb o", p=128)

    with tile.TileContext(nc) as tc:
        with (
            tc.tile_pool(name="const", bufs=1) as constp,
            tc.tile_pool(name="dram", bufs=1, space="DRAM") as dramp,
        ):
            cos_sb = constp.tile([128, T], bf16, name="cos_sb")
            sin_sb = constp.tile([128, T], bf16, name="sin_sb")
            mask_sb = constp.tile([128, 4, TCH], bf16, name="mask_sb")
            ones_sb = constp.tile([128, 128], bf16, name="ones_sb")
            nc.gpsimd.memset(mask_sb[:], 1.0)
            for jd in range(4):
                # keep 1.0 where  tq_rel - tk_rel - 128*jd >= 0  else 0
                nc.gpsimd.affine_select(
                    out=mask_sb[:, jd, :], in_=mask_sb[:, jd, :],
                    compare_op=mybir.AluOpType.is_ge, fill=0.0,
                    base=-128 * jd, pattern=[[1, TCH]], channel_multiplier=-1,
                )
            nc.gpsimd.memset(ones_sb[:], 1.0)

            # per (tq-half, local head) AllToAll bounce buffers
            a2a_in = [[dramp.tile([NCORES, 128, TQ], bf16,
                                  name=f"a2a_in{p}{h}") for h in range(HL)]
                      for p in range(2)]
            a2a_out = [[dramp.tile([NCORES, 128, TQ], bf16,
                                   name=f"a2a_out{p}{h}") for h in range(HL)]
                       for p in range(2)]

            with tc.tile_pool(name="qkv", bufs=1) as qkvp:
                qT = [[qkvp.tile([128, T], bf16, name=f"qT_{hl}_{b}")
                       for b in range(B)] for hl in range(HL)]
                kT = [[qkvp.tile([128, T], bf16, name=f"kT_{hl}_{b}")
                       for b in range(B)] for hl in range(HL)]
                V = [qkvp.tile([128, TB, 2 * HD], bf16, name=f"V_{b}")
                     for b in range(B)]

                # -------- Phase 1: QKV projection + RoPE ------------------
                with (
                    tc.tile_pool(name="wqk", bufs=1) as wqkp,
                    tc.tile_pool(name="xin", bufs=2) as xp,
                    tc.tile_pool(name="ps_qk", bufs=3, space="PSUM") as psqk,
                    tc.tile_pool(name="ps_v", bufs=2, space="PSUM") as psv,
                    tc.tile_pool(name="rope", bufs=3) as ropep,
                ):
                    wqk_sb = wqkp.tile([128, CB, 6 * HD], bf16,
                                       name="wqk_sb")
                    nc.sync.dma_start(wqk_sb[:, 0:CB // 2, :],
                                      wqkT_v[:, 0:CB // 2, :])
                    nc.sync.dma_start(wqk_sb[:, CB // 2:CB, :],
                                      wqkT_v[:, CB // 2:CB, :])

                    for ch in range(NCH):
                        b, cc = divmod(ch, CHB)
                        t0 = cc * TCH
                        xpan = xp.tile([128, CB, TCH], bf16, tag="xpan",
                                       name=f"xpan_{ch}")
                        # first panel rides the idle ACT HWDGE queue so it
                        # overlaps the weight load on the Sync queue
                        eng = nc.scalar if ch == 0 else nc.sync
                        for g in range(2):
                            eng.dma_start(
                                xpan[:, g * CB // 2:(g + 1) * CB // 2, :],
                                xT_v[:, g * CB // 2:(g + 1) * CB // 2,
                                     ch * TCH:(ch + 1) * TCH])
                        if ch == 0:
                            # behind the critical first weight/x loads
                            nc.sync.dma_start(cos_sb[:], cosT_d[:, :])
                            nc.sync.dma_start(sin_sb[:], sinS_d[:, :])

                        for f in range(4):  # q_h0 q_h1 k_h0 k_h1
                            ps = psqk.tile([128, TCH], f32, tag="qk",
                                           name=f"psqk_{ch}_{f}")
                            for cb in range(CB):
                                nc.tensor.matmul(
                                    ps[:],
                                    lhsT=wqk_sb[:, cb, f * 128:(f + 1) * 128],
                                    rhs=xpan[:, cb, :],
                                    start=(cb == 0), stop=(cb == CB - 1))
                            qraw = ropep.tile([128, TCH], bf16, tag="qraw",
                                              name=f"qraw_{ch}_{f}")
                            nc.scalar.copy(qraw[:], ps[:])
                            rot = ropep.tile([128, TCH], bf16, tag="rot",
                                             name=f"rot_{ch}_{f}")
                            nc.vector.stream_shuffle(rot[:], qraw[:],
                                                     mask=_SHUF)
                            t1 = ropep.tile([128, TCH], bf16, tag="t1",
                                            name=f"t1_{ch}_{f}")
                            nc.vector.tensor_mul(
                                t1[:], qraw[:], cos_sb[:, t0:t0 + TCH])
                            nc.vector.tensor_mul(
                                rot[:], rot[:], sin_sb[:, t0:t0 + TCH])
                            dest = (qT if f < 2 else kT)[f % 2][b]
                            nc.vector.tensor_add(
                                dest[:, t0:t0 + TCH], t1[:], rot[:])

                        for tb in range(TCH // 128):  # v
                            pv = psv.tile([128, 2 * HD], f32, tag="v",
                                          name=f"psv_{ch}_{tb}")
                            for cb in range(CB):
                                nc.tensor.matmul(
                                    pv[:],
                                    lhsT=xpan[:, cb, tb * 128:(tb + 1) * 128],
                                    rhs=wqk_sb[:, cb, 4 * 128:6 * 128],
                                    start=(cb == 0), stop=(cb == CB - 1))
                            nc.scalar.copy(V[b][:, cc * 4 + tb, :], pv[:])

                # -------- Phase 2: attention + resharding -----------------
                attnall_t = []
                wout_pre = {}
                with (
                    tc.tile_pool(name="attn", bufs=2) as attnp,
                    tc.tile_pool(name="wout", bufs=2) as woutp,
                ):
                  with (
                    tc.tile_pool(name="ps_st", bufs=2, space="PSUM") as psst,
                    tc.tile_pool(name="ps_acc", bufs=4, space="PSUM") as psacc,
                    tc.tile_pool(name="pexp", bufs=4) as pexpp,
                    tc.tile_pool(name="onorm", bufs=3) as onp,
                  ):
                    for part in range(2):
                        # layout [128, i(core), hl, t] == attnallT c order
                        attnall = attnp.tile([128, CB // 2, HL, TQ], bf16,
                                             tag="attnall",
                                             name=f"attnall_{part}")
                        attnall_t.append(attnall)
                        for hl in range(HL):
                            for tqc in range(part, CHB, 2):
                                for b in range(B):
                                    _attn_chunk(
                                        nc, mybir, psst, psacc, pexpp, onp,
                                        qT, kT, V, mask_sb, ones_sb,
                                        a2a_in[part][hl], b, hl, tqc,
                                        SCALE, Exp, f32, bf16)
                            # reshard this (half, head) while the rest of
                            # attention / the output projection runs
                            nc.gpsimd.collective_compute(
                                "AllToAll", mybir.AluOpType.bypass,
                                replica_groups=[list(range(NCORES))],
                                ins=[a2a_in[part][hl].opt()],
                                outs=[a2a_out[part][hl].opt()],
                            )
                            # critical post-collective load on the (idle)
                            # gpsimd queue, not stuck behind Sync DMAs
                            nc.gpsimd.dma_start(
                                attnall[:, :, hl, :],
                                a2a_out[part][hl].rearrange(
                                    "i p t -> p i t"))
                            if part == 0 and hl == 1:
                                for oc in range(2):
                                    w = woutp.tile(
                                        [128, CB, TCH], bf16, tag="wout",
                                        name=f"wout_0_{oc}")
                                    nc.gpsimd.dma_start(
                                        w[:],
                                        woutT_v[:, :,
                                                oc * TCH:(oc + 1) * TCH])
                                    wout_pre[(0, oc)] = w

                  # -------- Phase 3: output projection --------------------
                  with (
                    tc.tile_pool(name="ps_out", bufs=2, space="PSUM") as pso,
                    tc.tile_pool(name="o3", bufs=3) as o3p,
                  ):
                    last_mm = None
                    first_mm_p1 = None
                    for part in range(2):
                        attnall = attnall_t[part]
                        for oc in range(4):
                            if (part, oc) in wout_pre:
                                w = wout_pre[(part, oc)]
                            else:
                                w = woutp.tile([128, CB, TCH], bf16,
                                               tag="wout",
                                               name=f"wout_{part}_{oc}")
                                nc.sync.dma_start(
                                    w[:],
                                    woutT_v[:, :, oc * TCH:(oc + 1) * TCH])
                            for tb in range(TQ // 128):
                                po = pso.tile([128, TCH], f32, tag="out",
                                              name=f"po_{part}_{oc}_{tb}")
                                for cb in range(CB):
                                    mm = nc.tensor.matmul(
                                        po[:],
                                        lhsT=attnall[:, cb // 2, cb % 2,
                                                     tb * 128:(tb + 1) * 128],
                                        rhs=w[:, cb, :],
                                        start=(cb == 0),
                                        stop=(cb == CB - 1))
                                    if part == 1 and first_mm_p1 is None:
                                        first_mm_p1 = mm
                                    if part == 0:
                                        last_mm = mm
                                ot = o3p.tile([128, TCH], f32, tag="o3",
                                              name=f"ot_{part}_{oc}_{tb}")
                                nc.scalar.copy(ot[:], po[:])
                                nc.sync.dma_start(
                                    out_d[part * TQ + tb * 128:
                                          part * TQ + (tb + 1) * 128,
                                          oc * TCH:(oc + 1) * TCH],
                                    ot[:])
                    # keep the two out-proj halves in emission order on the
                    # PE so part 1 (gated on the later collectives) cannot
                    # starve part 0's remaining matmuls
                    tile.add_dep_helper(
                        first_mm_p1.ins, last_mm.ins, sync=False,
                        reason="outproj part order")

    nc.compile()
    return nc


def _attn_chunk(nc, mybir, psst, psacc, pexpp, onp, qT, kT, V, mask_sb,
                ones_sb, a2a_in_ph, b, hl, tqc, SCALE, Exp, f32, bf16):
    """One (batch, head, 512-query-chunk) of causal attention."""
    ntk = (tqc + 1) * (TCH // 128)
    npair = ntk // 2
    q_sl = qT[hl][b][:, tqc * TCH:(tqc + 1) * TCH]
    av = psacc.tile([128, TCH], f32, tag="acc", name=f"av_{b}_{hl}_{tqc}")
    ones_ps = psacc.tile([128, TCH], f32, tag="acc",
                         name=f"on_{b}_{hl}_{tqc}")
    pexp_t = {}

    def col0(j):
        """First valid tq column for tk-block j (causal: tq >= tk)."""
        jd = j - (TCH // 128) * tqc
        return 128 * jd if jd > 0 else 0

    def emit_pair(p):
        st = psst.tile([128, 2 * TCH], f32, tag="st",
                       name=f"st_{b}_{hl}_{tqc}_{p}")
        pe = pexpp.tile([128, 2 * TCH], bf16, tag="pexp",
                        name=f"pe_{b}_{hl}_{tqc}_{p}")
        for jj in range(2):
            j = 2 * p + jj
            c0 = col0(j)
            nc.tensor.matmul(
                st[:, jj * TCH + c0:(jj + 1) * TCH],
                lhsT=kT[hl][b][:, j * 128:(j + 1) * 128],
                rhs=q_sl[:, c0:TCH], start=True, stop=True)
        if col0(2 * p) == 0 and col0(2 * p + 1) == 0:
            nc.scalar.activation(pe[:], st[:], Exp, scale=SCALE)
        else:
            for jj in range(2):
                c0 = col0(2 * p + jj)
                nc.scalar.activation(
                    pe[:, jj * TCH + c0:(jj + 1) * TCH],
                    st[:, jj * TCH + c0:(jj + 1) * TCH], Exp, scale=SCALE)
        for jj in range(2):
            j = 2 * p + jj
            jd = j - (TCH // 128) * tqc
            if jd >= 0:  # diagonal block: causal mask on its triangle
                c0 = col0(j)
                sl = pe[:, jj * TCH + c0:(jj + 1) * TCH]
                nc.vector.tensor_mul(sl, sl, mask_sb[:, jd, c0:TCH])
        pexp_t[p] = pe

    emit_pair(0)
    for p in range(npair):
        if p + 1 < npair:
            emit_pair(p + 1)
        pe = pexp_t.pop(p)
        for jj in range(2):
            j = 2 * p + jj
            c0 = col0(j)
            sl = pe[:, jj * TCH + c0:(jj + 1) * TCH]
            first = j == 0  # always full width: sets has_written everywhere
            last = j == ntk - 1
            nc.tensor.matmul(ones_ps[:, c0:TCH], lhsT=ones_sb[:], rhs=sl,
                             start=first, stop=last, skip_group_check=True)
            nc.tensor.matmul(
                av[:, c0:TCH],
                lhsT=V[b][:, j, hl * 128:(hl + 1) * 128], rhs=sl,
                start=first, stop=last, skip_group_check=True)

    recip = onp.tile([128, TCH], f32, tag="recip", name=f"rc_{b}_{hl}_{tqc}")
    nc.vector.reciprocal_approx_fast(recip[:], ones_ps[:])
    oT = onp.tile([128, TCH], bf16, tag="oT", name=f"oT_{b}_{hl}_{tqc}")
    nc.vector.tensor_mul(oT[:], av[:], recip[:])
    dj = b * 2 + tqc // 2
    nc.sync.dma_start(a2a_in_ph[dj, :, :], oT[:])


def prep_inputs(x, cos, sin, w_qkv, w_out, T=2048):
    """Host-side shard/layout prep. Returns in_maps for the 8 cores."""
    TOK = B * T
    xT = np.ascontiguousarray(x.reshape(TOK, D).T).astype(BF16)
    cosT = np.ascontiguousarray(cos.T[_PERM, :]).astype(BF16)
    sinS = np.ascontiguousarray(sin.T[_PERM, :] * _SIGN[:, None]).astype(BF16)
    woutT = np.ascontiguousarray(w_out.T).astype(BF16)
    in_maps = []
    for c in range(NCORES):
        rows = []
        for sec in range(2):  # q, k (perm'd)
            for hl in range(HL):
                h = 2 * c + hl
                w = w_qkv[sec * D + h * HD:sec * D + (h + 1) * HD, :]
                rows.append(w[_PERM, :])
        for hl in range(HL):  # v natural
            h = 2 * c + hl
            rows.append(w_qkv[2 * D + h * HD:2 * D + (h + 1) * HD, :])
        wqkT = np.ascontiguousarray(np.concatenate(rows, 0).T).astype(BF16)
        in_maps.append({"xT": xT, "wqkT": wqkT, "woutT": woutT,
                        "cosT": cosT, "sinS": sinS})
    return in_maps


_NC_CACHE = {}


def _get_nc(T=2048):
    if T not in _NC_CACHE:
        _NC_CACHE[T] = build_nc(T)
    return _NC_CACHE[T]


def kernel(x, cos, sin, w_qkv, w_out):
    import concourse.bass_utils as bass_utils

    T = x.shape[1]
    x = np.asarray(x, np.float32)
    cos = np.asarray(cos, np.float32)
    sin = np.asarray(sin, np.float32)
    w_qkv = np.asarray(w_qkv, np.float32)
    w_out = np.asarray(w_out, np.float32)

    nc = _get_nc(T)
    in_maps = prep_inputs(x, cos, sin, w_qkv, w_out, T)
    res = bass_utils.run_bass_kernel_spmd(nc, in_maps,
                                          core_ids=list(range(NCORES)))
    THALF = T // 2
    full = np.empty((B, T, D), np.float32)
    for j in range(NCORES):
        b, hf = divmod(j, 2)
        full[b, hf * THALF:(hf + 1) * THALF, :] = res.results[j]["out"]
    return full



# revision 10
# speedup vs baseline: 1.2469x; 1.2469x over previous
"""Causal self-attention (B=4, T=2048, D=2048, H=16, HD=128) on 8 Trainium2
NeuronCores.

Sharding: Megatron-style tensor parallel over heads for QKV projection +
attention (2 heads per core), then on-device AllToAlls reshard from
head-parallel to token-parallel (core j owns tokens of batch j//2, half j%2)
for the output projection.  Host only slices/transposes weights, replicates
activations, and concatenates the 8 output shards.

fp8 (e4m3, DoubleRow perf mode = 2 contraction planes per matmul) carries the
error-tolerant matmuls; bf16 carve-outs protect the places softmax averaging
can't wash quantization noise out:
  - q/k/v projection: fp8 for token chunks >= 512 of each batch; the first
    512-token chunk stays bf16 (rows with few attention keys see q/k/v
    noise almost unaveraged).
  - AV + denominator matmuls: fp8 via fp8 exp(probs) and fp8 V for key
    blocks >= 256; the first 256 keys of each batch stay bf16.
  - scores and out-projection stay bf16 (out-proj weight noise is coherent
    in the output; scores fp8 would force a half-partition RoPE layout).
w_qkv is host-scaled by 16 so fp8's subnormal floor doesn't eat the
~N(0, D^-1/2) weights; the scale cancels exactly: 1/256 folds into the
exp scale and the softmax-denominator ones-vector is 16.0 (av16/den16).
exp uses bias -1.5 so fp8 pexp can't hit e4m3's 240 max.

Device layouts (fp32 PSUM accumulation everywhere):
  xT    [D, B*T]   x transposed (contraction dim on partitions)
  qT/kT [128, T]   per (local head, batch); d-order permuted so the RoPE
                   rotate-half partner sits 16 partitions away (within a
                   32-partition quadrant, reachable by DVE stream_shuffle).
                   Any consistent permutation of d leaves q.k unchanged.
  V     [T, 128]   natural d order (feeds AV matmul lhsT and out-proj order)
  S^T   [tk, tq]   scores transposed: the softmax sum over the partition dim
                   is a ones-matmul on the PE (output rows are the broadcast
                   sums for free); no max-subtraction needed (logits ~
                   N(0,1), bounded ~ +-6, exp can't overflow after bias).

The attention loop runs tq-half 0 (even 512-token chunks) then half 1, with
one AllToAll per (half, head) issued as soon as that head's chunks finish —
all four collectives overlap the remaining attention / output projection.
"""

import sys

for _p in ("/opt/trn_rl_repo", "/root/.axon_site/_ro/trn_rl_repo"):
    if _p not in sys.path:
        sys.path.insert(0, _p)

import numpy as np
import ml_dtypes

BF16 = ml_dtypes.bfloat16
FP8 = ml_dtypes.float8_e4m3

B = 4
D = 2048
H = 16
HD = 128
NCORES = 8
HL = 2           # heads per core
CB = D // 128    # contraction blocks
TCH = 512        # token chunk (matmul moving free dim)
WS = 16.0        # host-side w_qkv scale (fp8 subnormal avoidance)
EXP_BIAS = -4.0  # exp(l - 4): max causal logit ~8 on randn data; keeps
                 # fp8 pexp under e4m3's 240 (inf -> NaN otherwise)


def _perm128():
    """Partition order for q/k head dims: quadrant g holds dims
    [16g,16g+16) (lo) then [64+16g, 64+16g+16) (hi), so the rotate-half
    partner of partition p is p+-16 (same 32-partition quadrant)."""
    perm = np.zeros(128, np.int64)
    for p in range(128):
        g, i = divmod(p, 32)
        perm[p] = g * 16 + i if i < 16 else 64 + g * 16 + (i - 16)
    return perm


_PERM = _perm128()
_SHUF = [(i + 16) % 32 for i in range(32)]  # out[i] = in[(i+16)%32]
_SIGN = np.where(np.arange(128) % 32 < 16, -1.0, 1.0).astype(np.float32)


def build_nc(T=2048):
    import concourse.bacc as bacc
    import concourse.tile as tile
    import concourse.mybir as mybir

    f32 = mybir.dt.float32
    bf16 = mybir.dt.bfloat16
    f8 = mybir.dt.float8e4
    TOK = B * T
    THALF = T // 2
    TQ = THALF // 2           # tokens per (core, a2a part)
    NCH = TOK // TCH          # token chunks total
    CHB = T // TCH            # token chunks per batch
    TB = T // 128             # 128-token blocks per batch
    SCALE = float(HD) ** -0.5
    Exp = mybir.ActivationFunctionType.Exp
    DR = mybir.MatmulPerfMode.DoubleRow

    assert TQ == TCH, "A2A split layout assumes T == 2048"
    nc = bacc.Bacc("TRN2", target_bir_lowering=False, debug=False,
                   num_devices=NCORES)

    xT_d = nc.dram_tensor("xT", [D, TOK], bf16, kind="ExternalInput")
    x8_d = nc.dram_tensor("x8", [D, TOK], f8, kind="ExternalInput")
    wqkT_d = nc.dram_tensor("wqkT", [D, 6 * HD], bf16, kind="ExternalInput")
    wqk8_d = nc.dram_tensor("wqk8", [D, 6 * HD], f8, kind="ExternalInput")
    woutT_d = nc.dram_tensor("woutT", [D, D], bf16, kind="ExternalInput")
    cosT_d = nc.dram_tensor("cosT", [HD, T], bf16, kind="ExternalInput")
    sinS_d = nc.dram_tensor("sinS", [HD, T], bf16, kind="ExternalInput")
    out_d = nc.dram_tensor("out", [THALF, D], f32, kind="ExternalOutput")

    xT_v = xT_d.ap().rearrange("(cb p) t -> p cb t", p=128)
    x8_v = x8_d.ap().rearrange("(cb p) t -> p cb t", p=128)
    wqkT_v = wqkT_d.ap().rearrange("(cb p) f -> p cb f", p=128)
    wqk8_v = wqk8_d.ap().rearrange("(cb p) f -> p cb f", p=128)
    woutT_v = woutT_d.ap().rearrange("(cb p) o -> p cb o", p=128)

    with tile.TileContext(nc) as tc:
        with (
            tc.tile_pool(name="const", bufs=1) as constp,
            tc.tile_pool(name="dram", bufs=1, space="DRAM") as dramp,
        ):
            cos_sb = constp.tile([128, T], bf16, name="cos_sb")
            sin_sb = constp.tile([128, T], bf16, name="sin_sb")
            mask_sb = constp.tile([128, 4, TCH], bf16, name="mask_sb")
            mask8 = constp.tile([128, 4, TCH], f8, name="mask8")
            ones_sb = constp.tile([128, 128], bf16, name="ones_sb")
            ones8 = constp.tile([128, 2, 128], f8, name="ones8")
            ebias = constp.tile([128, 1], f32, name="ebias")
            nc.gpsimd.memset(ebias[:], EXP_BIAS)
            nc.gpsimd.memset(mask_sb[:], 1.0)
            for jd in range(4):
                # keep 1.0 where  tq_rel - tk_rel - 128*jd >= 0  else 0
                nc.gpsimd.affine_select(
                    out=mask_sb[:, jd, :], in_=mask_sb[:, jd, :],
                    compare_op=mybir.AluOpType.is_ge, fill=0.0,
                    base=-128 * jd, pattern=[[1, TCH]], channel_multiplier=-1,
                )
            nc.scalar.copy(mask8[:], mask_sb[:])
            # 16.0 folds the w_qkv host scale out of the softmax denominator
            nc.gpsimd.memset(ones_sb[:], WS)
            nc.gpsimd.memset(ones8[:], WS)

            # per (tq-half, local head) AllToAll bounce buffers
            a2a_in = [[dramp.tile([NCORES, 128, TQ], bf16,
                                  name=f"a2a_in{p}{h}") for h in range(HL)]
                      for p in range(2)]
            a2a_out = [[dramp.tile([NCORES, 128, TQ], bf16,
                                   name=f"a2a_out{p}{h}") for h in range(HL)]
                       for p in range(2)]

            with tc.tile_pool(name="qkv", bufs=1) as qkvp:
                qT = [[qkvp.tile([128, T], bf16, name=f"qT_{hl}_{b}")
                       for b in range(B)] for hl in range(HL)]
                kT = [[qkvp.tile([128, T], bf16, name=f"kT_{hl}_{b}")
                       for b in range(B)] for hl in range(HL)]
                # V: first two 128-token key blocks bf16, rest fp8
                Vb = [qkvp.tile([128, 2, 2 * HD], bf16, name=f"Vb_{b}")
                      for b in range(B)]
                V8 = [qkvp.tile([128, TB, 2 * HD], f8, name=f"V8_{b}")
                      for b in range(B)]

                # -------- Phase 1: QKV projection + RoPE ------------------
                with (
                    tc.tile_pool(name="wqk", bufs=1) as wqkp,
                    tc.tile_pool(name="xin", bufs=1) as xp,
                    tc.tile_pool(name="xin8", bufs=2) as xp8,
                    tc.tile_pool(name="ps_qk", bufs=3, space="PSUM") as psqk,
                    tc.tile_pool(name="ps_v", bufs=2, space="PSUM") as psv,
                    tc.tile_pool(name="rope", bufs=3) as ropep,
                ):
                    wqk_sb = wqkp.tile([128, CB, 6 * HD], bf16,
                                       name="wqk_sb")
                    wqk8_sb = wqkp.tile([128, CB, 6 * HD], f8,
                                        name="wqk8_sb")
                    nc.sync.dma_start(wqk_sb[:, 0:CB // 2, :],
                                      wqkT_v[:, 0:CB // 2, :])
                    nc.sync.dma_start(wqk_sb[:, CB // 2:CB, :],
                                      wqkT_v[:, CB // 2:CB, :])
                    # gpsimd HWDGE queue is idle in phase 1; keeps the fp8
                    # weight load off the scalar queue that carries the
                    # first x panel
                    nc.gpsimd.dma_start(wqk8_sb[:], wqk8_v[:])

                    for ch in range(NCH):
                        b, cc = divmod(ch, CHB)
                        t0 = cc * TCH
                        lo8 = cc > 0  # fp8 path for chunks past the first
                        if lo8:
                            xpan = xp8.tile([128, CB, TCH], f8, tag="xpan8",
                                            name=f"xpan8_{ch}")
                            src = x8_v
                        else:
                            xpan = xp.tile([128, CB, TCH], bf16, tag="xpan",
                                           name=f"xpan_{ch}")
                            src = xT_v
                        if ch == 0:
                            # first panel split in quarters across the idle
                            # ACT + DVE HWDGE queues so the first matmul's
                            # cb blocks land asap, overlapping the weight
                            # load on the Sync queue
                            for g in range(4):
                                eng = nc.scalar if g % 2 == 0 else nc.vector
                                eng.dma_start(
                                    xpan[:, g * CB // 4:(g + 1) * CB // 4, :],
                                    src[:, g * CB // 4:(g + 1) * CB // 4,
                                        0:TCH])
                        else:
                            for g in range(2):
                                nc.sync.dma_start(
                                    xpan[:, g * CB // 2:(g + 1) * CB // 2, :],
                                    src[:, g * CB // 2:(g + 1) * CB // 2,
                                        ch * TCH:(ch + 1) * TCH])
                        if ch == 0:
                            # behind the critical first weight/x loads
                            nc.sync.dma_start(cos_sb[:], cosT_d[:, :])
                            nc.sync.dma_start(sin_sb[:], sinS_d[:, :])

                        for f in range(4):  # q_h0 q_h1 k_h0 k_h1
                            ps = psqk.tile([128, TCH], f32, tag="qk",
                                           name=f"psqk_{ch}_{f}")
                            if lo8:
                                for cp in range(CB // 2):
                                    nc.tensor.matmul(
                                        ps[:],
                                        lhsT=wqk8_sb[:, 2 * cp:2 * cp + 2,
                                                     f * 128:(f + 1) * 128],
                                        rhs=xpan[:, 2 * cp:2 * cp + 2, :],
                                        start=(cp == 0),
                                        stop=(cp == CB // 2 - 1),
                                        perf_mode=DR)
                            else:
                                for cb in range(CB):
                                    nc.tensor.matmul(
                                        ps[:],
                                        lhsT=wqk_sb[:, cb,
                                                    f * 128:(f + 1) * 128],
                                        rhs=xpan[:, cb, :],
                                        start=(cb == 0), stop=(cb == CB - 1))
                            qraw = ropep.tile([128, TCH], bf16, tag="qraw",
                                              name=f"qraw_{ch}_{f}")
                            nc.scalar.copy(qraw[:], ps[:])
                            rot = ropep.tile([128, TCH], bf16, tag="rot",
                                             name=f"rot_{ch}_{f}")
                            nc.vector.stream_shuffle(rot[:], qraw[:],
                                                     mask=_SHUF)
                            t1 = ropep.tile([128, TCH], bf16, tag="t1",
                                            name=f"t1_{ch}_{f}")
                            nc.vector.tensor_mul(
                                t1[:], qraw[:], cos_sb[:, t0:t0 + TCH])
                            nc.vector.tensor_mul(
                                rot[:], rot[:], sin_sb[:, t0:t0 + TCH])
                            dest = (qT if f < 2 else kT)[f % 2][b]
                            nc.vector.tensor_add(
                                dest[:, t0:t0 + TCH], t1[:], rot[:])

                        for tb in range(TCH // 128):  # v
                            pv = psv.tile([128, 2 * HD], f32, tag="v",
                                          name=f"psv_{ch}_{tb}")
                            if lo8:
                                for cp in range(CB // 2):
                                    nc.tensor.matmul(
                                        pv[:],
                                        lhsT=xpan[:, 2 * cp:2 * cp + 2,
                                                  tb * 128:(tb + 1) * 128],
                                        rhs=wqk8_sb[:, 2 * cp:2 * cp + 2,
                                                    4 * 128:6 * 128],
                                        start=(cp == 0),
                                        stop=(cp == CB // 2 - 1),
                                        perf_mode=DR)
                            else:
                                for cb in range(CB):
                                    nc.tensor.matmul(
                                        pv[:],
                                        lhsT=xpan[:, cb,
                                                  tb * 128:(tb + 1) * 128],
                                        rhs=wqk_sb[:, cb, 4 * 128:6 * 128],
                                        start=(cb == 0), stop=(cb == CB - 1))
                            blk = cc * 4 + tb
                            if blk < 2:
                                nc.scalar.copy(Vb[b][:, blk, :], pv[:])
                            else:
                                nc.scalar.copy(V8[b][:, blk, :], pv[:])

                # -------- Phase 2: attention + resharding -----------------
                attnall_t = []
                wout_pre = {}
                with (
                    tc.tile_pool(name="attn", bufs=2) as attnp,
                    tc.tile_pool(name="wout", bufs=2) as woutp,
                ):
                  with (
                    tc.tile_pool(name="ps_st", bufs=2, space="PSUM") as psst,
                    tc.tile_pool(name="ps_acc", bufs=4, space="PSUM") as psacc,
                    tc.tile_pool(name="pexp", bufs=2) as pexpp,
                    tc.tile_pool(name="pexp8", bufs=4) as pexp8p,
                    tc.tile_pool(name="onorm", bufs=3) as onp,
                  ):
                    pools = (psst, psacc, pexpp, pexp8p, onp)
                    tiles = (qT, kT, Vb, V8, mask_sb, mask8, ones_sb, ones8,
                             ebias)
                    for part in range(2):
                        # layout [128, i(core), hl, t] == attnallT c order
                        attnall = attnp.tile([128, CB // 2, HL, TQ], bf16,
                                             tag="attnall",
                                             name=f"attnall_{part}")
                        attnall_t.append(attnall)
                        for hl in range(HL):
                            for tqc in range(part, CHB, 2):
                                for b in range(B):
                                    _attn_chunk(
                                        nc, mybir, pools, tiles,
                                        a2a_in[part][hl], b, hl, tqc,
                                        SCALE, Exp, f32, bf16)
                            # reshard this (half, head) while the rest of
                            # attention / the output projection runs
                            nc.gpsimd.collective_compute(
                                "AllToAll", mybir.AluOpType.bypass,
                                replica_groups=[list(range(NCORES))],
                                ins=[a2a_in[part][hl].opt()],
                                outs=[a2a_out[part][hl].opt()],
                            )
                            # critical post-collective load on the (idle)
                            # gpsimd queue, not stuck behind Sync DMAs
                            nc.gpsimd.dma_start(
                                attnall[:, :, hl, :],
                                a2a_out[part][hl].rearrange(
                                    "i p t -> p i t"))
                            if part == 0 and hl == 1:
                                for oc in range(2):
                                    w = woutp.tile(
                                        [128, CB, TCH], bf16, tag="wout",
                                        name=f"wout_0_{oc}")
                                    nc.gpsimd.dma_start(
                                        w[:],
                                        woutT_v[:, :,
                                                oc * TCH:(oc + 1) * TCH])
                                    wout_pre[(0, oc)] = w

                  # -------- Phase 3: output projection --------------------
                  with (
                    tc.tile_pool(name="ps_out", bufs=2, space="PSUM") as pso,
                    tc.tile_pool(name="o3", bufs=3) as o3p,
                  ):
                    last_mm = None
                    first_mm_p1 = None
                    for part in range(2):
                        attnall = attnall_t[part]
                        for oc in range(4):
                            if (part, oc) in wout_pre:
                                w = wout_pre[(part, oc)]
                            else:
                                w = woutp.tile([128, CB, TCH], bf16,
                                               tag="wout",
                                               name=f"wout_{part}_{oc}")
                                nc.sync.dma_start(
                                    w[:],
                                    woutT_v[:, :, oc * TCH:(oc + 1) * TCH])
                            for tb in range(TQ // 128):
                                po = pso.tile([128, TCH], f32, tag="out",
                                              name=f"po_{part}_{oc}_{tb}")
                                for cb in range(CB):
                                    mm = nc.tensor.matmul(
                                        po[:],
                                        lhsT=attnall[:, cb // 2, cb % 2,
                                                     tb * 128:(tb + 1) * 128],
                                        rhs=w[:, cb, :],
                                        start=(cb == 0),
                                        stop=(cb == CB - 1))
                                    if part == 1 and first_mm_p1 is None:
                                        first_mm_p1 = mm
                                    if part == 0:
                                        last_mm = mm
                                ot = o3p.tile([128, TCH], f32, tag="o3",
                                              name=f"ot_{part}_{oc}_{tb}")
                                nc.scalar.copy(ot[:], po[:])
                                nc.sync.dma_start(
                                    out_d[part * TQ + tb * 128:
                                          part * TQ + (tb + 1) * 128,
                                          oc * TCH:(oc + 1) * TCH],
                                    ot[:])
                    # keep the two out-proj halves in emission order on the
                    # PE so part 1 (gated on the later collectives) cannot
                    # starve part 0's remaining matmuls
                    tile.add_dep_helper(
                        first_mm_p1.ins, last_mm.ins, sync=False,
                        reason="outproj part order")

    nc.compile()
    return nc


def _attn_chunk(nc, mybir, pools, tiles, a2a_in_ph, b, hl, tqc,
                SCALE, Exp, f32, bf16):
    """One (batch, head, 512-query-chunk) of causal attention."""
    psst, psacc, pexpp, pexp8p, onp = pools
    qT, kT, Vb, V8, mask_sb, mask8, ones_sb, ones8, ebias = tiles
    f8 = mybir.dt.float8e4
    DRM = mybir.MatmulPerfMode.DoubleRow
    ntk = (tqc + 1) * (TCH // 128)
    npair = ntk // 2
    q_sl = qT[hl][b][:, tqc * TCH:(tqc + 1) * TCH]
    av = psacc.tile([128, TCH], f32, tag="acc", name=f"av_{b}_{hl}_{tqc}")
    ones_ps = psacc.tile([128, TCH], f32, tag="acc",
                         name=f"on_{b}_{hl}_{tqc}")
    pexp_t = {}
    ESC = SCALE / (WS * WS)  # undo the 16x q,k host scale inside exp

    def col0(j):
        """First valid tq column for tk-block j (causal: tq >= tk)."""
        jd = j - (TCH // 128) * tqc
        return 128 * jd if jd > 0 else 0

    def emit_pair(p):
        st = psst.tile([128, 2 * TCH], f32, tag="st",
                       name=f"st_{b}_{hl}_{tqc}_{p}")
        # fp8 pairs feed one DoubleRow AV matmul over the shared column
        # range [c0p, TCH); score/exp the odd block down to c0p too (the
        # mask zeroes its sub-diagonal strip) so no garbage PSUM is read
        c0p = col0(2 * p)
        for jj in range(2):
            j = 2 * p + jj
            c0 = col0(j) if p == 0 else c0p
            nc.tensor.matmul(
                st[:, jj * TCH + c0:(jj + 1) * TCH],
                lhsT=kT[hl][b][:, j * 128:(j + 1) * 128],
                rhs=q_sl[:, c0:TCH], start=True, stop=True)
        # pair 0 (first 256 keys) exponentiates to bf16 for the bf16 AV;
        # later pairs go straight to fp8 for the DoubleRow AV matmul
        if p == 0:
            pe = pexpp.tile([128, 2, TCH], bf16, tag="pexp",
                            name=f"pe_{b}_{hl}_{tqc}_{p}")
        else:
            pe = pexp8p.tile([128, 2, TCH], f8, tag="pexp8",
                            name=f"pe_{b}_{hl}_{tqc}_{p}")
        if c0p == 0 and col0(2 * p + 1) == 0:
            nc.scalar.activation(pe[:], st[:], Exp, scale=ESC, bias=ebias[:])
        else:
            for jj in range(2):
                c0 = col0(2 * p + jj) if p == 0 else c0p
                nc.scalar.activation(
                    pe[:, jj, c0:TCH],
                    st[:, jj * TCH + c0:(jj + 1) * TCH], Exp, scale=ESC,
                    bias=ebias[:])
        msk = mask_sb if p == 0 else mask8
        for jj in range(2):
            j = 2 * p + jj
            jd = j - (TCH // 128) * tqc
            if jd >= 0:  # diagonal block: causal mask on its triangle
                c0 = col0(j) if p == 0 else c0p
                sl = pe[:, jj, c0:TCH]
                nc.vector.tensor_mul(sl, sl, msk[:, jd, c0:TCH])
        pexp_t[p] = pe

    emit_pair(0)
    for p in range(npair):
        if p + 1 < npair:
            emit_pair(p + 1)
        pe = pexp_t.pop(p)
        if p == 0:
            for jj in range(2):
                j = jj
                c0 = col0(j)
                sl = pe[:, jj, c0:TCH]
                first = j == 0  # always full width: sets has_written
                last = j == ntk - 1
                nc.tensor.matmul(ones_ps[:, c0:TCH], lhsT=ones_sb[:], rhs=sl,
                                 start=first, stop=last,
                                 skip_group_check=True)
                nc.tensor.matmul(
                    av[:, c0:TCH],
                    lhsT=Vb[b][:, j, hl * 128:(hl + 1) * 128], rhs=sl,
                    start=first, stop=last, skip_group_check=True)
        else:
            c0p = col0(2 * p)
            last = 2 * p + 1 == ntk - 1
            sl = pe[:, :, c0p:TCH]
            nc.tensor.matmul(ones_ps[:, c0p:TCH], lhsT=ones8[:], rhs=sl,
                             start=False, stop=last, perf_mode=DRM,
                             skip_group_check=True)
            nc.tensor.matmul(
                av[:, c0p:TCH],
                lhsT=V8[b][:, 2 * p:2 * p + 2, hl * 128:(hl + 1) * 128],
                rhs=sl, start=False, stop=last, perf_mode=DRM,
                skip_group_check=True)

    recip = onp.tile([128, TCH], f32, tag="recip", name=f"rc_{b}_{hl}_{tqc}")
    nc.vector.reciprocal_approx_fast(recip[:], ones_ps[:])
    oT = onp.tile([128, TCH], bf16, tag="oT", name=f"oT_{b}_{hl}_{tqc}")
    nc.vector.tensor_mul(oT[:], av[:], recip[:])
    dj = b * 2 + tqc // 2
    nc.sync.dma_start(a2a_in_ph[dj, :, :], oT[:])


def prep_inputs(x, cos, sin, w_qkv, w_out, T=2048):
    """Host-side shard/layout prep. Returns in_maps for the 8 cores."""
    TOK = B * T
    xT = np.ascontiguousarray(x.reshape(TOK, D).T)
    xT_b = xT.astype(BF16)
    xT_8 = xT.astype(FP8)
    cosT = np.ascontiguousarray(cos.T[_PERM, :]).astype(BF16)
    sinS = np.ascontiguousarray(sin.T[_PERM, :] * _SIGN[:, None]).astype(BF16)
    woutT = np.ascontiguousarray(w_out.T).astype(BF16)
    wq16 = w_qkv * WS
    in_maps = []
    for c in range(NCORES):
        rows = []
        for sec in range(2):  # q, k (perm'd)
            for hl in range(HL):
                h = 2 * c + hl
                w = wq16[sec * D + h * HD:sec * D + (h + 1) * HD, :]
                rows.append(w[_PERM, :])
        for hl in range(HL):  # v natural
            h = 2 * c + hl
            rows.append(wq16[2 * D + h * HD:2 * D + (h + 1) * HD, :])
        wqkT = np.ascontiguousarray(np.concatenate(rows, 0).T)
        in_maps.append({"xT": xT_b, "x8": xT_8,
                        "wqkT": wqkT.astype(BF16),
                        "wqk8": wqkT.astype(FP8),
                        "woutT": woutT, "cosT": cosT, "sinS": sinS})
    return in_maps


_NC_CACHE = {}


def _get_nc(T=2048):
    if T not in _NC_CACHE:
        _NC_CACHE[T] = build_nc(T)
    return _NC_CACHE[T]


def kernel(x, cos, sin, w_qkv, w_out):
    import concourse.bass_utils as bass_utils

    T = x.shape[1]
    x = np.asarray(x, np.float32)
    cos = np.asarray(cos, np.float32)
    sin = np.asarray(sin, np.float32)
    w_qkv = np.asarray(w_qkv, np.float32)
    w_out = np.asarray(w_out, np.float32)

    nc = _get_nc(T)
    in_maps = prep_inputs(x, cos, sin, w_qkv, w_out, T)
    res = bass_utils.run_bass_kernel_spmd(nc, in_maps,
                                          core_ids=list(range(NCORES)))
    THALF = T // 2
    full = np.empty((B, T, D), np.float32)
    for j in range(NCORES):
        b, hf = divmod(j, 2)
        full[b, hf * THALF:(hf + 1) * THALF, :] = res.results[j]["out"]
    return full


# revision 16
# speedup vs baseline: 1.2614x; 1.0116x over previous
"""Causal self-attention (B=4, T=2048, D=2048, H=16, HD=128) on 8 Trainium2
NeuronCores.

Sharding: Megatron-style tensor parallel over heads for QKV projection +
attention (2 heads per core), then on-device AllToAlls reshard from
head-parallel to token-parallel (core j owns tokens of batch j//2, half j%2)
for the output projection.  Host only slices/transposes weights, replicates
activations, and concatenates the 8 output shards.

fp8 (e4m3, DoubleRow perf mode = 2 contraction planes per matmul) carries the
error-tolerant matmuls; bf16 carve-outs protect the places softmax averaging
can't wash quantization noise out:
  - q/k/v projection: fp8 for token chunks >= 512 of each batch; the first
    512-token chunk stays bf16 (rows with few attention keys see q/k/v
    noise almost unaveraged).
  - AV + denominator matmuls: fp8 via fp8 exp(probs) and fp8 V for key
    blocks >= 256; the first 256 keys of each batch stay bf16.
  - scores and out-projection stay bf16 (out-proj weight noise is coherent
    in the output; scores fp8 would force a half-partition RoPE layout).
w_qkv is host-scaled by 16 so fp8's subnormal floor doesn't eat the
~N(0, D^-1/2) weights; the scale cancels exactly: 1/256 folds into the
exp scale and the softmax-denominator ones-vector is 16.0 (av16/den16).
exp uses bias -1.5 so fp8 pexp can't hit e4m3's 240 max.

Device layouts (fp32 PSUM accumulation everywhere):
  xT    [D, B*T]   x transposed (contraction dim on partitions)
  qT/kT [128, T]   per (local head, batch); d-order permuted so the RoPE
                   rotate-half partner sits 16 partitions away (within a
                   32-partition quadrant, reachable by DVE stream_shuffle).
                   Any consistent permutation of d leaves q.k unchanged.
  V     [T, 128]   natural d order (feeds AV matmul lhsT and out-proj order)
  S^T   [tk, tq]   scores transposed: the softmax sum over the partition dim
                   is a ones-matmul on the PE (output rows are the broadcast
                   sums for free); no max-subtraction needed (logits ~
                   N(0,1), bounded ~ +-6, exp can't overflow after bias).

The attention loop runs tq-half 0 (even 512-token chunks) then half 1, with
one AllToAll per (half, head) issued as soon as that head's chunks finish —
all four collectives overlap the remaining attention / output projection.
"""

import sys

for _p in ("/opt/trn_rl_repo", "/root/.axon_site/_ro/trn_rl_repo"):
    if _p not in sys.path:
        sys.path.insert(0, _p)

import numpy as np
import ml_dtypes

BF16 = ml_dtypes.bfloat16
FP8 = ml_dtypes.float8_e4m3

B = 4
D = 2048
H = 16
HD = 128
NCORES = 8
HL = 2           # heads per core
CB = D // 128    # contraction blocks
TCH = 512        # token chunk (matmul moving free dim)
WS = 16.0        # host-side w_qkv scale (fp8 subnormal avoidance)
EXP_BIAS = -4.0  # exp(l - 4): max causal logit ~8 on randn data; keeps
                 # fp8 pexp under e4m3's 240 (inf -> NaN otherwise)


def _perm128():
    """Partition order for q/k head dims: quadrant g holds dims
    [16g,16g+16) (lo) then [64+16g, 64+16g+16) (hi), so the rotate-half
    partner of partition p is p+-16 (same 32-partition quadrant)."""
    perm = np.zeros(128, np.int64)
    for p in range(128):
        g, i = divmod(p, 32)
        perm[p] = g * 16 + i if i < 16 else 64 + g * 16 + (i - 16)
    return perm


_PERM = _perm128()
_SHUF = [(i + 16) % 32 for i in range(32)]  # out[i] = in[(i+16)%32]
_SIGN = np.where(np.arange(128) % 32 < 16, -1.0, 1.0).astype(np.float32)


def build_nc(T=2048):
    import concourse.bacc as bacc
    import concourse.tile as tile
    import concourse.mybir as mybir

    f32 = mybir.dt.float32
    bf16 = mybir.dt.bfloat16
    f8 = mybir.dt.float8e4
    TOK = B * T
    THALF = T // 2
    TQ = THALF // 2           # tokens per (core, a2a part)
    NCH = TOK // TCH          # token chunks total
    CHB = T // TCH            # token chunks per batch
    TB = T // 128             # 128-token blocks per batch
    SCALE = float(HD) ** -0.5
    Exp = mybir.ActivationFunctionType.Exp
    DR = mybir.MatmulPerfMode.DoubleRow

    assert TQ == TCH, "A2A split layout assumes T == 2048"
    nc = bacc.Bacc("TRN2", target_bir_lowering=False, debug=False,
                   num_devices=NCORES)

    xT_d = nc.dram_tensor("xT", [D, TOK], bf16, kind="ExternalInput")
    x8_d = nc.dram_tensor("x8", [D, TOK], f8, kind="ExternalInput")
    wqkT_d = nc.dram_tensor("wqkT", [D, 6 * HD], bf16, kind="ExternalInput")
    wqk8_d = nc.dram_tensor("wqk8", [D, 6 * HD], f8, kind="ExternalInput")
    woutT_d = nc.dram_tensor("woutT", [D, D], bf16, kind="ExternalInput")
    cosT_d = nc.dram_tensor("cosT", [HD, T], bf16, kind="ExternalInput")
    sinS_d = nc.dram_tensor("sinS", [HD, T], bf16, kind="ExternalInput")
    out_d = nc.dram_tensor("out", [THALF, D], f32, kind="ExternalOutput")

    xT_v = xT_d.ap().rearrange("(cb p) t -> p cb t", p=128)
    x8_v = x8_d.ap().rearrange("(cb p) t -> p cb t", p=128)
    wqkT_v = wqkT_d.ap().rearrange("(cb p) f -> p cb f", p=128)
    wqk8_v = wqk8_d.ap().rearrange("(cb p) f -> p cb f", p=128)
    woutT_v = woutT_d.ap().rearrange("(cb p) o -> p cb o", p=128)

    with tile.TileContext(nc) as tc:
        with (
            tc.tile_pool(name="const", bufs=1) as constp,
            tc.tile_pool(name="dram", bufs=1, space="DRAM") as dramp,
        ):
            cos_sb = constp.tile([128, T], bf16, name="cos_sb")
            sin_sb = constp.tile([128, T], bf16, name="sin_sb")
            mask_sb = constp.tile([128, 4, TCH], bf16, name="mask_sb")
            mask8 = constp.tile([128, 4, TCH], f8, name="mask8")
            ones_sb = constp.tile([128, 128], bf16, name="ones_sb")
            ones8 = constp.tile([128, 2, 128], f8, name="ones8")
            ebias = constp.tile([128, 1], f32, name="ebias")
            nc.gpsimd.memset(ebias[:], EXP_BIAS)
            nc.gpsimd.memset(mask_sb[:], 1.0)
            for jd in range(4):
                # keep 1.0 where  tq_rel - tk_rel - 128*jd >= 0  else 0
                nc.gpsimd.affine_select(
                    out=mask_sb[:, jd, :], in_=mask_sb[:, jd, :],
                    compare_op=mybir.AluOpType.is_ge, fill=0.0,
                    base=-128 * jd, pattern=[[1, TCH]], channel_multiplier=-1,
                )
            nc.scalar.copy(mask8[:], mask_sb[:])
            # 16.0 folds the w_qkv host scale out of the softmax denominator
            nc.gpsimd.memset(ones_sb[:], WS)
            nc.gpsimd.memset(ones8[:], WS)

            # per (tq-half, local head) AllToAll bounce buffers
            a2a_in = [[dramp.tile([NCORES, 128, TQ], bf16,
                                  name=f"a2a_in{p}{h}") for h in range(HL)]
                      for p in range(2)]
            a2a_out = [[dramp.tile([NCORES, 128, TQ], bf16,
                                   name=f"a2a_out{p}{h}") for h in range(HL)]
                       for p in range(2)]

            with tc.tile_pool(name="qkv", bufs=1) as qkvp:
                qT = [[qkvp.tile([128, T], bf16, name=f"qT_{hl}_{b}")
                       for b in range(B)] for hl in range(HL)]
                kT = [[qkvp.tile([128, T], bf16, name=f"kT_{hl}_{b}")
                       for b in range(B)] for hl in range(HL)]
                # V: first two 128-token key blocks bf16, rest fp8
                Vb = [qkvp.tile([128, 2, 2 * HD], bf16, name=f"Vb_{b}")
                      for b in range(B)]
                V8 = [qkvp.tile([128, TB, 2 * HD], f8, name=f"V8_{b}")
                      for b in range(B)]

                # -------- Phase 1: QKV projection + RoPE ------------------
                with (
                    tc.tile_pool(name="wqk", bufs=1) as wqkp,
                    tc.tile_pool(name="xin", bufs=1) as xp,
                    tc.tile_pool(name="xin8", bufs=2) as xp8,
                    tc.tile_pool(name="ps_qk", bufs=3, space="PSUM") as psqk,
                    tc.tile_pool(name="ps_v", bufs=2, space="PSUM") as psv,
                    tc.tile_pool(name="rope", bufs=3) as ropep,
                ):
                    wqk_sb = wqkp.tile([128, CB, 6 * HD], bf16,
                                       name="wqk_sb")
                    wqk8_sb = wqkp.tile([128, CB, 6 * HD], f8,
                                        name="wqk8_sb")
                    nc.sync.dma_start(wqk_sb[:, 0:CB // 2, :],
                                      wqkT_v[:, 0:CB // 2, :])
                    nc.sync.dma_start(wqk_sb[:, CB // 2:CB, :],
                                      wqkT_v[:, CB // 2:CB, :])

                    for ch in range(NCH):
                        b, cc = divmod(ch, CHB)
                        t0 = cc * TCH
                        lo8 = cc > 0  # fp8 path for chunks past the first
                        if lo8:
                            xpan = xp8.tile([128, CB, TCH], f8, tag="xpan8",
                                            name=f"xpan8_{ch}")
                            src = x8_v
                        else:
                            xpan = xp.tile([128, CB, TCH], bf16, tag="xpan",
                                           name=f"xpan_{ch}")
                            src = xT_v
                        if ch == 0:
                            # first panel split in quarters across the idle
                            # ACT + GpSimd HWDGE queues so the first
                            # matmul's cb blocks land asap, overlapping the
                            # weight load on the Sync queue
                            for g in range(4):
                                eng = nc.scalar if g % 2 == 0 else nc.gpsimd
                                eng.dma_start(
                                    xpan[:, g * CB // 4:(g + 1) * CB // 4, :],
                                    src[:, g * CB // 4:(g + 1) * CB // 4,
                                        0:TCH])
                        else:
                            for g in range(2):
                                nc.sync.dma_start(
                                    xpan[:, g * CB // 2:(g + 1) * CB // 2, :],
                                    src[:, g * CB // 2:(g + 1) * CB // 2,
                                        ch * TCH:(ch + 1) * TCH])
                        if ch == 0:
                            # behind the critical first weight/x loads;
                            # the fp8 weights (first needed by chunk 1)
                            # ride the gpsimd queue behind ch0's x quarters
                            nc.gpsimd.dma_start(wqk8_sb[:], wqk8_v[:])
                            nc.sync.dma_start(cos_sb[:], cosT_d[:, :])
                            nc.sync.dma_start(sin_sb[:], sinS_d[:, :])

                        for f in range(4):  # q_h0 q_h1 k_h0 k_h1
                            ps = psqk.tile([128, TCH], f32, tag="qk",
                                           name=f"psqk_{ch}_{f}")
                            if lo8:
                                for cp in range(CB // 2):
                                    nc.tensor.matmul(
                                        ps[:],
                                        lhsT=wqk8_sb[:, 2 * cp:2 * cp + 2,
                                                     f * 128:(f + 1) * 128],
                                        rhs=xpan[:, 2 * cp:2 * cp + 2, :],
                                        start=(cp == 0),
                                        stop=(cp == CB // 2 - 1),
                                        perf_mode=DR)
                            else:
                                for cb in range(CB):
                                    nc.tensor.matmul(
                                        ps[:],
                                        lhsT=wqk_sb[:, cb,
                                                    f * 128:(f + 1) * 128],
                                        rhs=xpan[:, cb, :],
                                        start=(cb == 0), stop=(cb == CB - 1))
                            qraw = ropep.tile([128, TCH], bf16, tag="qraw",
                                              name=f"qraw_{ch}_{f}")
                            nc.scalar.copy(qraw[:], ps[:])
                            rot = ropep.tile([128, TCH], bf16, tag="rot",
                                             name=f"rot_{ch}_{f}")
                            nc.vector.stream_shuffle(rot[:], qraw[:],
                                                     mask=_SHUF)
                            t1 = ropep.tile([128, TCH], bf16, tag="t1",
                                            name=f"t1_{ch}_{f}")
                            nc.vector.tensor_mul(
                                t1[:], qraw[:], cos_sb[:, t0:t0 + TCH])
                            nc.vector.tensor_mul(
                                rot[:], rot[:], sin_sb[:, t0:t0 + TCH])
                            dest = (qT if f < 2 else kT)[f % 2][b]
                            nc.vector.tensor_add(
                                dest[:, t0:t0 + TCH], t1[:], rot[:])

                        for tb in range(TCH // 128):  # v
                            pv = psv.tile([128, 2 * HD], f32, tag="v",
                                          name=f"psv_{ch}_{tb}")
                            if lo8:
                                for cp in range(CB // 2):
                                    nc.tensor.matmul(
                                        pv[:],
                                        lhsT=xpan[:, 2 * cp:2 * cp + 2,
                                                  tb * 128:(tb + 1) * 128],
                                        rhs=wqk8_sb[:, 2 * cp:2 * cp + 2,
                                                    4 * 128:6 * 128],
                                        start=(cp == 0),
                                        stop=(cp == CB // 2 - 1),
                                        perf_mode=DR)
                            else:
                                for cb in range(CB):
                                    nc.tensor.matmul(
                                        pv[:],
                                        lhsT=xpan[:, cb,
                                                  tb * 128:(tb + 1) * 128],
                                        rhs=wqk_sb[:, cb, 4 * 128:6 * 128],
                                        start=(cb == 0), stop=(cb == CB - 1))
                            blk = cc * 4 + tb
                            if blk < 2:
                                nc.scalar.copy(Vb[b][:, blk, :], pv[:])
                            else:
                                nc.scalar.copy(V8[b][:, blk, :], pv[:])

                # -------- Phase 2: attention + resharding -----------------
                attnall_t = []
                wout_pre = {}
                with (
                    tc.tile_pool(name="attn", bufs=2) as attnp,
                    tc.tile_pool(name="wout", bufs=2) as woutp,
                ):
                  with (
                    tc.tile_pool(name="ps_st", bufs=2, space="PSUM") as psst,
                    tc.tile_pool(name="ps_acc", bufs=4, space="PSUM") as psacc,
                    tc.tile_pool(name="pexp", bufs=2) as pexpp,
                    tc.tile_pool(name="pexp8", bufs=4) as pexp8p,
                    tc.tile_pool(name="onorm", bufs=3) as onp,
                  ):
                    pools = (psst, psacc, pexpp, pexp8p, onp)
                    tiles = (qT, kT, Vb, V8, mask_sb, mask8, ones_sb, ones8,
                             ebias)
                    for part in range(2):
                        # layout [128, i(core), hl, t] == attnallT c order
                        attnall_t.append(
                            attnp.tile([128, CB // 2, HL, TQ], bf16,
                                       tag="attnall", name=f"attnall_{part}"))
                    # software-pipelined over all 32 chunks: the next
                    # chunk's first scores+exp are emitted before this
                    # chunk's last AV so the PE never waits out the
                    # scores->exp->mask->AV latency at a chunk boundary
                    descs = [(part, hl, tqc, b)
                             for part in range(2) for hl in range(HL)
                             for tqc in range(part, CHB, 2)
                             for b in range(B)]
                    states = [None] * len(descs)

                    def ensure_emit(i, p):
                        if states[i] is None:
                            part, hl, tqc, b = descs[i]
                            states[i] = _mk_attn_state(
                                nc, mybir, pools, tiles, b, hl, tqc,
                                SCALE, Exp, f32, bf16)
                        _attn_emit(nc, mybir, pools, tiles, states[i], p)

                    ensure_emit(0, 0)
                    grp = len(descs) // (2 * HL)  # chunks per (part, hl)
                    for i, (part, hl, tqc, b) in enumerate(descs):
                        stt = states[i]
                        for p in range(stt["npair"]):
                            if p + 1 < stt["npair"]:
                                ensure_emit(i, p + 1)
                            elif i + 1 < len(descs):
                                ensure_emit(i + 1, 0)
                            _attn_consume(nc, mybir, pools, tiles, stt, p)
                        _attn_finalize(nc, pools, stt, a2a_in[part][hl])
                        states[i] = None
                        if (i + 1) % grp:
                            continue
                        # (part, hl) group done: reshard it while the rest
                        # of attention / the output projection runs
                        nc.gpsimd.collective_compute(
                            "AllToAll", mybir.AluOpType.bypass,
                            replica_groups=[list(range(NCORES))],
                            ins=[a2a_in[part][hl].opt()],
                            outs=[a2a_out[part][hl].opt()],
                        )
                        # critical post-collective load on the (idle)
                        # gpsimd queue, not stuck behind Sync DMAs
                        nc.gpsimd.dma_start(
                            attnall_t[part][:, :, hl, :],
                            a2a_out[part][hl].rearrange("i p t -> p i t"))
                        if part == 0 and hl == 1:
                            for oc in range(2):
                                w = woutp.tile(
                                    [128, CB, TCH], bf16, tag="wout",
                                    name=f"wout_0_{oc}")
                                nc.gpsimd.dma_start(
                                    w[:],
                                    woutT_v[:, :, oc * TCH:(oc + 1) * TCH])
                                wout_pre[(0, oc)] = w

                  # -------- Phase 3: output projection --------------------
                  with (
                    tc.tile_pool(name="ps_out", bufs=2, space="PSUM") as pso,
                    tc.tile_pool(name="o3", bufs=3) as o3p,
                  ):
                    last_mm = None
                    first_mm_p1 = None
                    for part in range(2):
                        attnall = attnall_t[part]
                        for oc in range(4):
                            if (part, oc) in wout_pre:
                                w = wout_pre[(part, oc)]
                            else:
                                w = woutp.tile([128, CB, TCH], bf16,
                                               tag="wout",
                                               name=f"wout_{part}_{oc}")
                                nc.sync.dma_start(
                                    w[:],
                                    woutT_v[:, :, oc * TCH:(oc + 1) * TCH])
                            for tb in range(TQ // 128):
                                po = pso.tile([128, TCH], f32, tag="out",
                                              name=f"po_{part}_{oc}_{tb}")
                                for cb in range(CB):
                                    mm = nc.tensor.matmul(
                                        po[:],
                                        lhsT=attnall[:, cb // 2, cb % 2,
                                                     tb * 128:(tb + 1) * 128],
                                        rhs=w[:, cb, :],
                                        start=(cb == 0),
                                        stop=(cb == CB - 1))
                                    if part == 1 and first_mm_p1 is None:
                                        first_mm_p1 = mm
                                    if part == 0:
                                        last_mm = mm
                                ot = o3p.tile([128, TCH], f32, tag="o3",
                                              name=f"ot_{part}_{oc}_{tb}")
                                nc.scalar.copy(ot[:], po[:])
                                nc.sync.dma_start(
                                    out_d[part * TQ + tb * 128:
                                          part * TQ + (tb + 1) * 128,
                                          oc * TCH:(oc + 1) * TCH],
                                    ot[:])
                    # keep the two out-proj halves in emission order on the
                    # PE so part 1 (gated on the later collectives) cannot
                    # starve part 0's remaining matmuls
                    tile.add_dep_helper(
                        first_mm_p1.ins, last_mm.ins, sync=False,
                        reason="outproj part order")

    nc.compile()
    return nc


def _col0(j, tqc):
    """First valid tq column for tk-block j (causal: tq >= tk)."""
    jd = j - (TCH // 128) * tqc
    return 128 * jd if jd > 0 else 0


def _mk_attn_state(nc, mybir, pools, tiles, b, hl, tqc, SCALE, Exp, f32,
                   bf16):
    qT = tiles[0]
    return {
        "b": b, "hl": hl, "tqc": tqc,
        "ntk": (tqc + 1) * (TCH // 128),
        "npair": (tqc + 1) * (TCH // 128) // 2,
        "q_sl": qT[hl][b][:, tqc * TCH:(tqc + 1) * TCH],
        "av": None, "ones_ps": None, "pexp": {},
        "Exp": Exp, "f32": f32, "bf16": bf16,
        "esc": SCALE / (WS * WS),  # undo the 16x q,k host scale in exp
    }


def _attn_emit(nc, mybir, pools, tiles, stt, p):
    """Scores + exp + causal mask for tk-block pair p of one chunk."""
    psst, psacc, pexpp, pexp8p, onp = pools
    qT, kT, Vb, V8, mask_sb, mask8, ones_sb, ones8, ebias = tiles
    f8 = mybir.dt.float8e4
    b, hl, tqc = stt["b"], stt["hl"], stt["tqc"]
    bf16, Exp = stt["bf16"], stt["Exp"]
    st = psst.tile([128, 2, TCH], stt["f32"], tag="st",
                   name=f"st_{b}_{hl}_{tqc}_{p}")
    # fp8 pairs feed one DoubleRow AV matmul over the shared column
    # range [c0p, TCH); score/exp the odd block down to c0p too (the
    # mask zeroes its sub-diagonal strip) so no garbage PSUM is read
    c0p = _col0(2 * p, tqc)
    for jj in range(2):
        j = 2 * p + jj
        c0 = _col0(j, tqc) if p == 0 else c0p
        nc.tensor.matmul(
            st[:, jj, c0:TCH],
            lhsT=kT[hl][b][:, j * 128:(j + 1) * 128],
            rhs=stt["q_sl"][:, c0:TCH], start=True, stop=True)
    # pair 0 (first 256 keys) exponentiates to bf16 for the bf16 AV;
    # later pairs go straight to fp8 for the DoubleRow AV matmul
    if p == 0:
        pe = pexpp.tile([128, 2, TCH], bf16, tag="pexp",
                        name=f"pe_{b}_{hl}_{tqc}_{p}")
    else:
        pe = pexp8p.tile([128, 2, TCH], f8, tag="pexp8",
                         name=f"pe_{b}_{hl}_{tqc}_{p}")
    if p == 0 and _col0(1, tqc) != 0:
        # tqc==0 pair 0: the two blocks have different written ranges
        for jj in range(2):
            c0 = _col0(jj, tqc)
            nc.scalar.activation(
                pe[:, jj, c0:TCH], st[:, jj, c0:TCH], Exp,
                scale=stt["esc"], bias=ebias[:])
    else:
        # one strided activation covers both blocks' [c0p, TCH) range
        # (amortizes the ACT engine's ~300-cycle per-instruction overhead)
        nc.scalar.activation(pe[:, :, c0p:TCH], st[:, :, c0p:TCH], Exp,
                             scale=stt["esc"], bias=ebias[:])
    msk = mask_sb if p == 0 else mask8
    for jj in range(2):
        j = 2 * p + jj
        jd = j - (TCH // 128) * tqc
        if jd >= 0:  # diagonal block: causal mask on its triangle
            c0 = _col0(j, tqc) if p == 0 else c0p
            sl = pe[:, jj, c0:TCH]
            nc.vector.tensor_mul(sl, sl, msk[:, jd, c0:TCH])
    stt["pexp"][p] = pe


def _attn_consume(nc, mybir, pools, tiles, stt, p):
    """AV + denominator matmuls for tk-block pair p of one chunk."""
    psst, psacc, pexpp, pexp8p, onp = pools
    qT, kT, Vb, V8, mask_sb, mask8, ones_sb, ones8, ebias = tiles
    DRM = mybir.MatmulPerfMode.DoubleRow
    b, hl, tqc, ntk = stt["b"], stt["hl"], stt["tqc"], stt["ntk"]
    if stt["av"] is None:
        stt["av"] = psacc.tile([128, TCH], stt["f32"], tag="acc",
                               name=f"av_{b}_{hl}_{tqc}")
        stt["ones_ps"] = psacc.tile([128, TCH], stt["f32"], tag="acc",
                                    name=f"on_{b}_{hl}_{tqc}")
    av, ones_ps = stt["av"], stt["ones_ps"]
    pe = stt["pexp"].pop(p)
    if p == 0:
        for jj in range(2):
            c0 = _col0(jj, tqc)
            sl = pe[:, jj, c0:TCH]
            first = jj == 0  # always full width: sets has_written
            last = jj == ntk - 1
            nc.tensor.matmul(ones_ps[:, c0:TCH], lhsT=ones_sb[:], rhs=sl,
                             start=first, stop=last, skip_group_check=True)
            nc.tensor.matmul(
                av[:, c0:TCH],
                lhsT=Vb[b][:, jj, hl * 128:(hl + 1) * 128], rhs=sl,
                start=first, stop=last, skip_group_check=True)
    else:
        c0p = _col0(2 * p, tqc)
        last = 2 * p + 1 == ntk - 1
        sl = pe[:, :, c0p:TCH]
        nc.tensor.matmul(ones_ps[:, c0p:TCH], lhsT=ones8[:], rhs=sl,
                         start=False, stop=last, perf_mode=DRM,
                         skip_group_check=True)
        nc.tensor.matmul(
            av[:, c0p:TCH],
            lhsT=V8[b][:, 2 * p:2 * p + 2, hl * 128:(hl + 1) * 128],
            rhs=sl, start=False, stop=last, perf_mode=DRM,
            skip_group_check=True)


def _attn_finalize(nc, pools, stt, a2a_in_ph):
    """Normalize by the softmax denominator and stage for the AllToAll."""
    psst, psacc, pexpp, pexp8p, onp = pools
    b, hl, tqc = stt["b"], stt["hl"], stt["tqc"]
    recip = onp.tile([128, TCH], stt["f32"], tag="recip",
                     name=f"rc_{b}_{hl}_{tqc}")
    nc.vector.reciprocal_approx_fast(recip[:], stt["ones_ps"][:])
    oT = onp.tile([128, TCH], stt["bf16"], tag="oT",
                  name=f"oT_{b}_{hl}_{tqc}")
    nc.vector.tensor_mul(oT[:], stt["av"][:], recip[:])
    dj = b * 2 + tqc // 2
    nc.sync.dma_start(a2a_in_ph[dj, :, :], oT[:])


def prep_inputs(x, cos, sin, w_qkv, w_out, T=2048):
    """Host-side shard/layout prep. Returns in_maps for the 8 cores."""
    TOK = B * T
    xT = np.ascontiguousarray(x.reshape(TOK, D).T)
    xT_b = xT.astype(BF16)
    xT_8 = xT.astype(FP8)
    cosT = np.ascontiguousarray(cos.T[_PERM, :]).astype(BF16)
    sinS = np.ascontiguousarray(sin.T[_PERM, :] * _SIGN[:, None]).astype(BF16)
    woutT = np.ascontiguousarray(w_out.T).astype(BF16)
    wq16 = w_qkv * WS
    in_maps = []
    for c in range(NCORES):
        rows = []
        for sec in range(2):  # q, k (perm'd)
            for hl in range(HL):
                h = 2 * c + hl
                w = wq16[sec * D + h * HD:sec * D + (h + 1) * HD, :]
                rows.append(w[_PERM, :])
        for hl in range(HL):  # v natural
            h = 2 * c + hl
            rows.append(wq16[2 * D + h * HD:2 * D + (h + 1) * HD, :])
        wqkT = np.ascontiguousarray(np.concatenate(rows, 0).T)
        in_maps.append({"xT": xT_b, "x8": xT_8,
                        "wqkT": wqkT.astype(BF16),
                        "wqk8": wqkT.astype(FP8),
                        "woutT": woutT, "cosT": cosT, "sinS": sinS})
    return in_maps


_NC_CACHE = {}


def _get_nc(T=2048):
    if T not in _NC_CACHE:
        _NC_CACHE[T] = build_nc(T)
    return _NC_CACHE[T]


def kernel(x, cos, sin, w_qkv, w_out):
    import concourse.bass_utils as bass_utils

    T = x.shape[1]
    x = np.asarray(x, np.float32)
    cos = np.asarray(cos, np.float32)
    sin = np.asarray(sin, np.float32)
    w_qkv = np.asarray(w_qkv, np.float32)
    w_out = np.asarray(w_out, np.float32)

    nc = _get_nc(T)
    in_maps = prep_inputs(x, cos, sin, w_qkv, w_out, T)
    res = bass_utils.run_bass_kernel_spmd(nc, in_maps,
                                          core_ids=list(range(NCORES)))
    THALF = T // 2
    full = np.empty((B, T, D), np.float32)
    for j in range(NCORES):
        b, hf = divmod(j, 2)
        full[b, hf * THALF:(hf + 1) * THALF, :] = res.results[j]["out"]
    return full


# revision 22
# speedup vs baseline: 1.3046x; 1.0342x over previous
"""Causal self-attention (B=4, T=2048, D=2048, H=16, HD=128) on 8 Trainium2
NeuronCores.

Sharding: Megatron-style tensor parallel over heads for QKV projection +
attention (2 heads per core), then on-device AllToAlls reshard from
head-parallel to token-parallel (core j owns tokens of batch j//2, half j%2)
for the output projection.  Host only slices/transposes weights, replicates
activations, and concatenates the 8 output shards.

fp8 (e4m3, DoubleRow perf mode = 2 contraction planes per matmul) carries the
error-tolerant matmuls; bf16 carve-outs protect the places softmax averaging
can't wash quantization noise out:
  - q/k/v projection: fp8 for token chunks >= 512 of each batch; the first
    512-token chunk stays bf16 (rows with few attention keys see q/k/v
    noise almost unaveraged).
  - AV + denominator matmuls: fp8 via fp8 exp(probs) and fp8 V for key
    blocks >= 256; the first 256 keys of each batch stay bf16.
  - scores and out-projection stay bf16 (out-proj weight noise is coherent
    in the output; scores fp8 would force a half-partition RoPE layout).
w_qkv is host-scaled by 16 so fp8's subnormal floor doesn't eat the
~N(0, D^-1/2) weights; the scale cancels exactly: 1/256 folds into the
exp scale and the softmax-denominator ones-vector is 16.0 (av16/den16).
exp uses bias -1.5 so fp8 pexp can't hit e4m3's 240 max.

Device layouts (fp32 PSUM accumulation everywhere):
  xT    [D, B*T]   x transposed (contraction dim on partitions)
  qT/kT [128, T]   per (local head, batch); d-order permuted so the RoPE
                   rotate-half partner sits 16 partitions away (within a
                   32-partition quadrant, reachable by DVE stream_shuffle).
                   Any consistent permutation of d leaves q.k unchanged.
  V     [T, 128]   natural d order (feeds AV matmul lhsT and out-proj order)
  S^T   [tk, tq]   scores transposed: the softmax sum over the partition dim
                   is a ones-matmul on the PE (output rows are the broadcast
                   sums for free); no max-subtraction needed (logits ~
                   N(0,1), bounded ~ +-6, exp can't overflow after bias).

The attention loop runs tq-half 0 (even 512-token chunks) then half 1, with
one AllToAll per (half, head) issued as soon as that head's chunks finish —
all four collectives overlap the remaining attention / output projection.
"""

import sys

for _p in ("/opt/trn_rl_repo", "/root/.axon_site/_ro/trn_rl_repo"):
    if _p not in sys.path:
        sys.path.insert(0, _p)

import numpy as np
import ml_dtypes

BF16 = ml_dtypes.bfloat16
FP8 = ml_dtypes.float8_e4m3

B = 4
D = 2048
H = 16
HD = 128
NCORES = 8
HL = 2           # heads per core
CB = D // 128    # contraction blocks
TCH = 512        # token chunk (matmul moving free dim)
WS = 16.0        # host-side w_qkv scale (fp8 subnormal avoidance)
EXP_BIAS = -4.0  # exp(l - 4): max causal logit ~8 on randn data; keeps
                 # fp8 pexp under e4m3's 240 (inf -> NaN otherwise)


def _perm128():
    """Partition order for q/k head dims: quadrant g holds dims
    [16g,16g+16) (lo) then [64+16g, 64+16g+16) (hi), so the rotate-half
    partner of partition p is p+-16 (same 32-partition quadrant)."""
    perm = np.zeros(128, np.int64)
    for p in range(128):
        g, i = divmod(p, 32)
        perm[p] = g * 16 + i if i < 16 else 64 + g * 16 + (i - 16)
    return perm


_PERM = _perm128()
_SHUF = [(i + 16) % 32 for i in range(32)]  # out[i] = in[(i+16)%32]
_SIGN = np.where(np.arange(128) % 32 < 16, -1.0, 1.0).astype(np.float32)


def build_nc(T=2048):
    import concourse.bacc as bacc
    import concourse.tile as tile
    import concourse.mybir as mybir

    f32 = mybir.dt.float32
    bf16 = mybir.dt.bfloat16
    f8 = mybir.dt.float8e4
    TOK = B * T
    THALF = T // 2
    TQ = THALF // 2           # tokens per (core, a2a part)
    NCH = TOK // TCH          # token chunks total
    CHB = T // TCH            # token chunks per batch
    TB = T // 128             # 128-token blocks per batch
    SCALE = float(HD) ** -0.5
    Exp = mybir.ActivationFunctionType.Exp
    DR = mybir.MatmulPerfMode.DoubleRow

    assert TQ == TCH, "A2A split layout assumes T == 2048"
    nc = bacc.Bacc("TRN2", target_bir_lowering=False, debug=False,
                   num_devices=NCORES)

    xT_d = nc.dram_tensor("xT", [D, TOK], bf16, kind="ExternalInput")
    x8_d = nc.dram_tensor("x8", [D, TOK], f8, kind="ExternalInput")
    wqkT_d = nc.dram_tensor("wqkT", [D, 6 * HD], bf16, kind="ExternalInput")
    wqk8_d = nc.dram_tensor("wqk8", [D, 6 * HD], f8, kind="ExternalInput")
    woutT_d = nc.dram_tensor("woutT", [D, D], bf16, kind="ExternalInput")
    cosT_d = nc.dram_tensor("cosT", [HD, T], bf16, kind="ExternalInput")
    sinS_d = nc.dram_tensor("sinS", [HD, T], bf16, kind="ExternalInput")
    out_d = nc.dram_tensor("out", [THALF, D], f32, kind="ExternalOutput")

    xT_v = xT_d.ap().rearrange("(cb p) t -> p cb t", p=128)
    x8_v = x8_d.ap().rearrange("(cb p) t -> p cb t", p=128)
    wqkT_v = wqkT_d.ap().rearrange("(cb p) f -> p cb f", p=128)
    wqk8_v = wqk8_d.ap().rearrange("(cb p) f -> p cb f", p=128)
    woutT_v = woutT_d.ap().rearrange("(cb p) o -> p cb o", p=128)

    with tile.TileContext(nc) as tc:
        with (
            tc.tile_pool(name="const", bufs=1) as constp,
            tc.tile_pool(name="dram", bufs=1, space="DRAM") as dramp,
        ):
            cos_sb = constp.tile([128, T], bf16, name="cos_sb")
            sin_sb = constp.tile([128, T], bf16, name="sin_sb")
            mask_sb = constp.tile([128, 4, TCH], bf16, name="mask_sb")
            mask8 = constp.tile([128, 4, TCH], f8, name="mask8")
            ones_sb = constp.tile([128, 128], bf16, name="ones_sb")
            ones8 = constp.tile([128, 2, 128], f8, name="ones8")
            ebias = constp.tile([128, 1], f32, name="ebias")
            nc.gpsimd.memset(ebias[:], EXP_BIAS)
            nc.gpsimd.memset(mask_sb[:], 1.0)
            for jd in range(4):
                # keep 1.0 where  tq_rel - tk_rel - 128*jd >= 0  else 0
                nc.gpsimd.affine_select(
                    out=mask_sb[:, jd, :], in_=mask_sb[:, jd, :],
                    compare_op=mybir.AluOpType.is_ge, fill=0.0,
                    base=-128 * jd, pattern=[[1, TCH]], channel_multiplier=-1,
                )
            nc.scalar.copy(mask8[:], mask_sb[:])
            # 16.0 folds the w_qkv host scale out of the softmax denominator
            nc.gpsimd.memset(ones_sb[:], WS)
            nc.gpsimd.memset(ones8[:], WS)

            # per (tq-half, local head) AllToAll bounce buffers
            a2a_in = [[dramp.tile([NCORES, 128, TQ], bf16,
                                  name=f"a2a_in{p}{h}") for h in range(HL)]
                      for p in range(2)]
            a2a_out = [[dramp.tile([NCORES, 128, TQ], bf16,
                                   name=f"a2a_out{p}{h}") for h in range(HL)]
                       for p in range(2)]

            with tc.tile_pool(name="qkv", bufs=1) as qkvp:
                qT = [[qkvp.tile([128, T], bf16, name=f"qT_{hl}_{b}")
                       for b in range(B)] for hl in range(HL)]
                kT = [[qkvp.tile([128, T], bf16, name=f"kT_{hl}_{b}")
                       for b in range(B)] for hl in range(HL)]
                # V: first two 128-token key blocks bf16, rest fp8
                Vb = [qkvp.tile([128, 2, 2 * HD], bf16, name=f"Vb_{b}")
                      for b in range(B)]
                V8 = [qkvp.tile([128, TB, 2 * HD], f8, name=f"V8_{b}")
                      for b in range(B)]

                # -------- Phase 1: QKV projection + RoPE ------------------
                with (
                    tc.tile_pool(name="wqk", bufs=1) as wqkp,
                    tc.tile_pool(name="xin", bufs=1) as xp,
                    tc.tile_pool(name="xin8", bufs=2) as xp8,
                    tc.tile_pool(name="ps_qk", bufs=3, space="PSUM") as psqk,
                    tc.tile_pool(name="ps_v", bufs=2, space="PSUM") as psv,
                    tc.tile_pool(name="rope", bufs=3) as ropep,
                ):
                    wqk_sb = wqkp.tile([128, CB, 6 * HD], bf16,
                                       name="wqk_sb")
                    wqk8_sb = wqkp.tile([128, CB, 6 * HD], f8,
                                        name="wqk8_sb")
                    # tiny first piece so the first matmul's weights land
                    # in ~1us; the rest streams behind it
                    nc.sync.dma_start(wqk_sb[:, 0:2, :], wqkT_v[:, 0:2, :])
                    nc.sync.dma_start(wqk_sb[:, 2:CB // 2, :],
                                      wqkT_v[:, 2:CB // 2, :])
                    nc.sync.dma_start(wqk_sb[:, CB // 2:CB, :],
                                      wqkT_v[:, CB // 2:CB, :])

                    for ch in range(NCH):
                        b, cc = divmod(ch, CHB)
                        t0 = cc * TCH
                        lo8 = cc > 0  # fp8 path for chunks past the first
                        if lo8:
                            xpan = xp8.tile([128, CB, TCH], f8, tag="xpan8",
                                            name=f"xpan8_{ch}")
                            src = x8_v
                        else:
                            xpan = xp.tile([128, CB, TCH], bf16, tag="xpan",
                                           name=f"xpan_{ch}")
                            src = xT_v
                        if ch == 0:
                            # first panel split in pieces across the idle
                            # ACT + GpSimd HWDGE queues so the first
                            # matmul's cb blocks land asap, overlapping the
                            # weight load on the Sync queue
                            for g, (c0, c1) in enumerate(
                                    [(0, 1), (1, 4), (4, 8), (8, 12),
                                     (12, 16)]):
                                eng = nc.scalar if g % 2 == 0 else nc.gpsimd
                                eng.dma_start(xpan[:, c0:c1, :],
                                              src[:, c0:c1, 0:TCH])
                        else:
                            for g in range(2):
                                nc.sync.dma_start(
                                    xpan[:, g * CB // 2:(g + 1) * CB // 2, :],
                                    src[:, g * CB // 2:(g + 1) * CB // 2,
                                        ch * TCH:(ch + 1) * TCH])
                        if ch == 0:
                            # behind the critical first weight/x loads;
                            # the fp8 weights (first needed by chunk 1)
                            # ride the gpsimd queue behind ch0's x quarters
                            nc.gpsimd.dma_start(wqk8_sb[:], wqk8_v[:])
                            nc.sync.dma_start(cos_sb[:], cosT_d[:, :])
                            nc.sync.dma_start(sin_sb[:], sinS_d[:, :])

                        for f in range(4):  # q_h0 q_h1 k_h0 k_h1
                            ps = psqk.tile([128, TCH], f32, tag="qk",
                                           name=f"psqk_{ch}_{f}")
                            if lo8:
                                for cp in range(CB // 2):
                                    nc.tensor.matmul(
                                        ps[:],
                                        lhsT=wqk8_sb[:, 2 * cp:2 * cp + 2,
                                                     f * 128:(f + 1) * 128],
                                        rhs=xpan[:, 2 * cp:2 * cp + 2, :],
                                        start=(cp == 0),
                                        stop=(cp == CB // 2 - 1),
                                        perf_mode=DR)
                            else:
                                for cb in range(CB):
                                    nc.tensor.matmul(
                                        ps[:],
                                        lhsT=wqk_sb[:, cb,
                                                    f * 128:(f + 1) * 128],
                                        rhs=xpan[:, cb, :],
                                        start=(cb == 0), stop=(cb == CB - 1))
                            qraw = ropep.tile([128, TCH], bf16, tag="qraw",
                                              name=f"qraw_{ch}_{f}")
                            nc.scalar.copy(qraw[:], ps[:])
                            rot = ropep.tile([128, TCH], bf16, tag="rot",
                                             name=f"rot_{ch}_{f}")
                            nc.vector.stream_shuffle(rot[:], qraw[:],
                                                     mask=_SHUF)
                            t1 = ropep.tile([128, TCH], bf16, tag="t1",
                                            name=f"t1_{ch}_{f}")
                            nc.vector.tensor_mul(
                                t1[:], qraw[:], cos_sb[:, t0:t0 + TCH])
                            nc.vector.tensor_mul(
                                rot[:], rot[:], sin_sb[:, t0:t0 + TCH])
                            dest = (qT if f < 2 else kT)[f % 2][b]
                            nc.vector.tensor_add(
                                dest[:, t0:t0 + TCH], t1[:], rot[:])

                        for tb in range(TCH // 128):  # v
                            pv = psv.tile([128, 2 * HD], f32, tag="v",
                                          name=f"psv_{ch}_{tb}")
                            if lo8:
                                for cp in range(CB // 2):
                                    nc.tensor.matmul(
                                        pv[:],
                                        lhsT=xpan[:, 2 * cp:2 * cp + 2,
                                                  tb * 128:(tb + 1) * 128],
                                        rhs=wqk8_sb[:, 2 * cp:2 * cp + 2,
                                                    4 * 128:6 * 128],
                                        start=(cp == 0),
                                        stop=(cp == CB // 2 - 1),
                                        perf_mode=DR)
                            else:
                                for cb in range(CB):
                                    nc.tensor.matmul(
                                        pv[:],
                                        lhsT=xpan[:, cb,
                                                  tb * 128:(tb + 1) * 128],
                                        rhs=wqk_sb[:, cb, 4 * 128:6 * 128],
                                        start=(cb == 0), stop=(cb == CB - 1))
                            blk = cc * 4 + tb
                            if blk < 2:
                                nc.scalar.copy(Vb[b][:, blk, :], pv[:])
                            else:
                                nc.scalar.copy(V8[b][:, blk, :], pv[:])

                # -------- Phase 2: attention + resharding -----------------
                attnall_t = []
                wout_pre = {}
                with (
                    tc.tile_pool(name="attn", bufs=2) as attnp,
                    tc.tile_pool(name="wout", bufs=2) as woutp,
                ):
                  with (
                    tc.tile_pool(name="ps_st", bufs=2, space="PSUM") as psst,
                    tc.tile_pool(name="ps_acc", bufs=4, space="PSUM") as psacc,
                    tc.tile_pool(name="pexp", bufs=2) as pexpp,
                    tc.tile_pool(name="pexp8", bufs=4) as pexp8p,
                    tc.tile_pool(name="onorm", bufs=3) as onp,
                  ):
                    pools = (psst, psacc, pexpp, pexp8p, onp)
                    tiles = (qT, kT, Vb, V8, mask_sb, mask8, ones_sb, ones8,
                             ebias)
                    for part in range(2):
                        # layout [128, i(core), hl, t] == attnallT c order
                        attnall_t.append(
                            attnp.tile([128, CB // 2, HL, TQ], bf16,
                                       tag="attnall", name=f"attnall_{part}"))
                    # software-pipelined over all 32 chunks: the next
                    # chunk's first scores+exp are emitted before this
                    # chunk's last AV so the PE never waits out the
                    # scores->exp->mask->AV latency at a chunk boundary
                    # part 1 first: its collectives fire mid-attention, and
                    # part 0's last collective hides behind part 1's
                    # out-projection (which runs first in phase 3)
                    descs = [(part, hl, tqc, b)
                             for part in (1, 0) for hl in range(HL)
                             for tqc in range(part, CHB, 2)
                             for b in range(B)]
                    states = [None] * len(descs)

                    def ensure_emit(i, p):
                        if states[i] is None:
                            part, hl, tqc, b = descs[i]
                            states[i] = _mk_attn_state(
                                nc, mybir, pools, tiles, b, hl, tqc,
                                SCALE, Exp, f32, bf16)
                        _attn_emit(nc, mybir, pools, tiles, states[i], p)

                    ensure_emit(0, 0)
                    grp = len(descs) // (2 * HL)  # chunks per (part, hl)
                    for i, (part, hl, tqc, b) in enumerate(descs):
                        stt = states[i]
                        for p in range(stt["npair"]):
                            if p + 1 < stt["npair"]:
                                ensure_emit(i, p + 1)
                            elif i + 1 < len(descs):
                                ensure_emit(i + 1, 0)
                            _attn_consume(nc, mybir, pools, tiles, stt, p)
                        _attn_finalize(nc, pools, stt, a2a_in[part][hl])
                        states[i] = None
                        if (i + 1) % grp:
                            continue
                        # (part, hl) group done: reshard it while the rest
                        # of attention / the output projection runs
                        nc.gpsimd.collective_compute(
                            "AllToAll", mybir.AluOpType.bypass,
                            replica_groups=[list(range(NCORES))],
                            ins=[a2a_in[part][hl].opt()],
                            outs=[a2a_out[part][hl].opt()],
                        )
                        # critical post-collective load on the (idle)
                        # gpsimd queue, not stuck behind Sync DMAs
                        nc.gpsimd.dma_start(
                            attnall_t[part][:, :, hl, :],
                            a2a_out[part][hl].rearrange("i p t -> p i t"))
                        if part == 1 and hl == 1:
                            for oc in range(2):
                                w = woutp.tile(
                                    [128, CB, TCH], bf16, tag="wout",
                                    name=f"wout_1_{oc}")
                                nc.gpsimd.dma_start(
                                    w[:],
                                    woutT_v[:, :, oc * TCH:(oc + 1) * TCH])
                                wout_pre[(1, oc)] = w

                  # -------- Phase 3: output projection --------------------
                  with (
                    tc.tile_pool(name="ps_out", bufs=2, space="PSUM") as pso,
                    tc.tile_pool(name="o3", bufs=3) as o3p,
                  ):
                    last_mm = None
                    first_mm_p1 = None
                    for part in (1, 0):
                        attnall = attnall_t[part]
                        for oc in range(4):
                            if (part, oc) in wout_pre:
                                w = wout_pre[(part, oc)]
                            else:
                                w = woutp.tile([128, CB, TCH], bf16,
                                               tag="wout",
                                               name=f"wout_{part}_{oc}")
                                nc.sync.dma_start(
                                    w[:],
                                    woutT_v[:, :, oc * TCH:(oc + 1) * TCH])
                            for tb in range(TQ // 128):
                                po = pso.tile([128, TCH], f32, tag="out",
                                              name=f"po_{part}_{oc}_{tb}")
                                for cb in range(CB):
                                    mm = nc.tensor.matmul(
                                        po[:],
                                        lhsT=attnall[:, cb // 2, cb % 2,
                                                     tb * 128:(tb + 1) * 128],
                                        rhs=w[:, cb, :],
                                        start=(cb == 0),
                                        stop=(cb == CB - 1))
                                    if part == 0 and first_mm_p1 is None:
                                        first_mm_p1 = mm
                                    if part == 1:
                                        last_mm = mm
                                ot = o3p.tile([128, TCH], f32, tag="o3",
                                              name=f"ot_{part}_{oc}_{tb}")
                                nc.scalar.copy(ot[:], po[:])
                                nc.sync.dma_start(
                                    out_d[part * TQ + tb * 128:
                                          part * TQ + (tb + 1) * 128,
                                          oc * TCH:(oc + 1) * TCH],
                                    ot[:])
                    # keep the two out-proj halves in emission order on the
                    # PE so part 1 (gated on the later collectives) cannot
                    # starve part 0's remaining matmuls
                    tile.add_dep_helper(
                        first_mm_p1.ins, last_mm.ins, sync=False,
                        reason="outproj part order")

    nc.compile()
    return nc


def _col0(j, tqc):
    """First valid tq column for tk-block j (causal: tq >= tk)."""
    jd = j - (TCH // 128) * tqc
    return 128 * jd if jd > 0 else 0


def _mk_attn_state(nc, mybir, pools, tiles, b, hl, tqc, SCALE, Exp, f32,
                   bf16):
    qT = tiles[0]
    return {
        "b": b, "hl": hl, "tqc": tqc,
        "ntk": (tqc + 1) * (TCH // 128),
        "npair": (tqc + 1) * (TCH // 128) // 2,
        "q_sl": qT[hl][b][:, tqc * TCH:(tqc + 1) * TCH],
        "av": None, "ones_ps": None, "pexp": {},
        "Exp": Exp, "f32": f32, "bf16": bf16,
        "esc": SCALE / (WS * WS),  # undo the 16x q,k host scale in exp
    }


def _attn_emit(nc, mybir, pools, tiles, stt, p):
    """Scores + exp + causal mask for tk-block pair p of one chunk."""
    psst, psacc, pexpp, pexp8p, onp = pools
    qT, kT, Vb, V8, mask_sb, mask8, ones_sb, ones8, ebias = tiles
    f8 = mybir.dt.float8e4
    b, hl, tqc = stt["b"], stt["hl"], stt["tqc"]
    bf16, Exp = stt["bf16"], stt["Exp"]
    st = psst.tile([128, 2, TCH], stt["f32"], tag="st",
                   name=f"st_{b}_{hl}_{tqc}_{p}")
    # fp8 pairs feed one DoubleRow AV matmul over the shared column
    # range [c0p, TCH); score/exp the odd block down to c0p too (the
    # mask zeroes its sub-diagonal strip) so no garbage PSUM is read
    c0p = _col0(2 * p, tqc)
    for jj in range(2):
        j = 2 * p + jj
        c0 = _col0(j, tqc) if p == 0 else c0p
        nc.tensor.matmul(
            st[:, jj, c0:TCH],
            lhsT=kT[hl][b][:, j * 128:(j + 1) * 128],
            rhs=stt["q_sl"][:, c0:TCH], start=True, stop=True)
    # pair 0 (first 256 keys) exponentiates to bf16 for the bf16 AV;
    # later pairs go straight to fp8 for the DoubleRow AV matmul
    if p == 0:
        pe = pexpp.tile([128, 2, TCH], bf16, tag="pexp",
                        name=f"pe_{b}_{hl}_{tqc}_{p}")
    else:
        pe = pexp8p.tile([128, 2, TCH], f8, tag="pexp8",
                         name=f"pe_{b}_{hl}_{tqc}_{p}")
    if p == 0 and _col0(1, tqc) != 0:
        # tqc==0 pair 0: the two blocks have different written ranges
        for jj in range(2):
            c0 = _col0(jj, tqc)
            nc.scalar.activation(
                pe[:, jj, c0:TCH], st[:, jj, c0:TCH], Exp,
                scale=stt["esc"], bias=ebias[:])
    else:
        # one strided activation covers both blocks' [c0p, TCH) range
        # (amortizes the ACT engine's ~300-cycle per-instruction overhead)
        nc.scalar.activation(pe[:, :, c0p:TCH], st[:, :, c0p:TCH], Exp,
                             scale=stt["esc"], bias=ebias[:])
    msk = mask_sb if p == 0 else mask8
    for jj in range(2):
        j = 2 * p + jj
        jd = j - (TCH // 128) * tqc
        if jd >= 0:  # diagonal block: causal mask on its triangle
            c0 = _col0(j, tqc) if p == 0 else c0p
            sl = pe[:, jj, c0:TCH]
            nc.vector.tensor_mul(sl, sl, msk[:, jd, c0:TCH])
    stt["pexp"][p] = pe


def _attn_consume(nc, mybir, pools, tiles, stt, p):
    """AV + denominator matmuls for tk-block pair p of one chunk."""
    psst, psacc, pexpp, pexp8p, onp = pools
    qT, kT, Vb, V8, mask_sb, mask8, ones_sb, ones8, ebias = tiles
    DRM = mybir.MatmulPerfMode.DoubleRow
    b, hl, tqc, ntk = stt["b"], stt["hl"], stt["tqc"], stt["ntk"]
    if stt["av"] is None:
        stt["av"] = psacc.tile([128, TCH], stt["f32"], tag="acc",
                               name=f"av_{b}_{hl}_{tqc}")
        stt["ones_ps"] = psacc.tile([128, TCH], stt["f32"], tag="acc",
                                    name=f"on_{b}_{hl}_{tqc}")
    av, ones_ps = stt["av"], stt["ones_ps"]
    pe = stt["pexp"].pop(p)
    if p == 0:
        for jj in range(2):
            c0 = _col0(jj, tqc)
            sl = pe[:, jj, c0:TCH]
            first = jj == 0  # always full width: sets has_written
            last = jj == ntk - 1
            nc.tensor.matmul(ones_ps[:, c0:TCH], lhsT=ones_sb[:], rhs=sl,
                             start=first, stop=last, skip_group_check=True)
            nc.tensor.matmul(
                av[:, c0:TCH],
                lhsT=Vb[b][:, jj, hl * 128:(hl + 1) * 128], rhs=sl,
                start=first, stop=last, skip_group_check=True)
    else:
        c0p = _col0(2 * p, tqc)
        last = 2 * p + 1 == ntk - 1
        sl = pe[:, :, c0p:TCH]
        nc.tensor.matmul(ones_ps[:, c0p:TCH], lhsT=ones8[:], rhs=sl,
                         start=False, stop=last, perf_mode=DRM,
                         skip_group_check=True)
        nc.tensor.matmul(
            av[:, c0p:TCH],
            lhsT=V8[b][:, 2 * p:2 * p + 2, hl * 128:(hl + 1) * 128],
            rhs=sl, start=False, stop=last, perf_mode=DRM,
            skip_group_check=True)


def _attn_finalize(nc, pools, stt, a2a_in_ph):
    """Normalize by the softmax denominator and stage for the AllToAll."""
    psst, psacc, pexpp, pexp8p, onp = pools
    b, hl, tqc = stt["b"], stt["hl"], stt["tqc"]
    recip = onp.tile([128, TCH], stt["f32"], tag="recip",
                     name=f"rc_{b}_{hl}_{tqc}")
    nc.vector.reciprocal_approx_fast(recip[:], stt["ones_ps"][:])
    oT = onp.tile([128, TCH], stt["bf16"], tag="oT",
                  name=f"oT_{b}_{hl}_{tqc}")
    nc.vector.tensor_mul(oT[:], stt["av"][:], recip[:])
    dj = b * 2 + tqc // 2
    nc.sync.dma_start(a2a_in_ph[dj, :, :], oT[:])


def prep_inputs(x, cos, sin, w_qkv, w_out, T=2048):
    """Host-side shard/layout prep. Returns in_maps for the 8 cores."""
    TOK = B * T
    xT = np.ascontiguousarray(x.reshape(TOK, D).T)
    xT_b = xT.astype(BF16)
    xT_8 = xT.astype(FP8)
    cosT = np.ascontiguousarray(cos.T[_PERM, :]).astype(BF16)
    sinS = np.ascontiguousarray(sin.T[_PERM, :] * _SIGN[:, None]).astype(BF16)
    woutT = np.ascontiguousarray(w_out.T).astype(BF16)
    wq16 = w_qkv * WS
    in_maps = []
    for c in range(NCORES):
        rows = []
        for sec in range(2):  # q, k (perm'd)
            for hl in range(HL):
                h = 2 * c + hl
                w = wq16[sec * D + h * HD:sec * D + (h + 1) * HD, :]
                rows.append(w[_PERM, :])
        for hl in range(HL):  # v natural
            h = 2 * c + hl
            rows.append(wq16[2 * D + h * HD:2 * D + (h + 1) * HD, :])
        wqkT = np.ascontiguousarray(np.concatenate(rows, 0).T)
        in_maps.append({"xT": xT_b, "x8": xT_8,
                        "wqkT": wqkT.astype(BF16),
                        "wqk8": wqkT.astype(FP8),
                        "woutT": woutT, "cosT": cosT, "sinS": sinS})
    return in_maps


_NC_CACHE = {}


def _get_nc(T=2048):
    if T not in _NC_CACHE:
        _NC_CACHE[T] = build_nc(T)
    return _NC_CACHE[T]


def kernel(x, cos, sin, w_qkv, w_out):
    import concourse.bass_utils as bass_utils

    T = x.shape[1]
    x = np.asarray(x, np.float32)
    cos = np.asarray(cos, np.float32)
    sin = np.asarray(sin, np.float32)
    w_qkv = np.asarray(w_qkv, np.float32)
    w_out = np.asarray(w_out, np.float32)

    nc = _get_nc(T)
    in_maps = prep_inputs(x, cos, sin, w_qkv, w_out, T)
    res = bass_utils.run_bass_kernel_spmd(nc, in_maps,
                                          core_ids=list(range(NCORES)))
    THALF = T // 2
    full = np.empty((B, T, D), np.float32)
    for j in range(NCORES):
        b, hf = divmod(j, 2)
        full[b, hf * THALF:(hf + 1) * THALF, :] = res.results[j]["out"]
    return full
